# revision 20
# baseline (speedup 1.0000x reference)
"""Fused causal multi-head attention block on 8 Trainium2 NeuronCores.

Problem (GPT-2 style attention, B=2, S=2048, D=1024, H=16, hd=64):
    qkv = x @ w_attn + b_attn ; split q,k,v ; per-head causal softmax(q k^T / 8) v
    out = attn_out @ w_proj + b_proj

Sharding: data parallel on batch (2) x tensor parallel on heads (4 groups of 4
heads). Core c -> batch c//4, head group c%4. Each core computes a partial
[S, D] output (its heads' slice of w_proj rows); host sums the 4 partials per
batch and adds b_proj.

Per-core kernel layout tricks:
- scores are computed TRANSPOSED (scoresT[key, query]) so the softmax
  denominator falls out of the attn@v matmul by appending a ones-column to v:
  [v | 1]^T @ exp(scoresT) yields the unnormalized output and the per-query
  denominator in one PSUM accumulation.
- matmul inputs are fp16 (full PE rate + fast weight loads); all accumulation
  is fp32 in PSUM. exp(s/8) is in [0, ~13], well inside fp16 range.
- back-to-back matmuls accumulating into the SAME PSUM region serialize on
  the array drain (~175ns each). All contraction chains (qkv, proj) are
  emitted as interleaved PAIRS of chains targeting two PSUM regions, so one
  chain's fill overlaps the other's drain.
- causal masking: fully-masked blocks are skipped via restricted matmul
  widths; diagonal blocks get their masked triangle zeroed on the (otherwise
  idle) GpSimd engine, keeping both the PE and the vector engine out of the
  score->attnv chain.
- x is DMA'd in 4 seq-quarters so QKV chunk c only waits for quarter c;
  weight DMAs are ordered ahead of the x quarter they gate. Small consts go
  out on the gpsimd SWDGE ring in parallel with the sync ring.
- emission is chunk-pipelined: QKV chunk c+1, proj of completed chunks, and
  the previous pair's normalization are spread as PE filler between the
  exp-paced attnv groups, so the PE has dense matmul work while ScalarE runs
  exp. Two fillers are popped before the first attnv group to cover the
  ScalarE backlog at pair transitions.
- the two per-pair score matmuls (K=64 at partition bases 0/64) row-tile into
  the PE array concurrently; the two norm broadcast matmuls (M=64 at output
  bases 0/64) col-tile concurrently.
"""

import sys

sys.path.insert(0, "/opt/trn_rl_repo")

import numpy as np

import concourse.bass as bass
import concourse.mybir as mybir
import concourse.tile as tile
from concourse import bacc
from concourse.bass_utils import run_bass_kernel_spmd

F32 = mybir.dt.float32
F16 = mybir.dt.float16
AFT = mybir.ActivationFunctionType

B, S, D, H, HD = 2, 2048, 1024, 16, 64
NCORES = 8
HPC = 4            # heads per core
CH = HPC * HD      # 256 channels per core
VW = HD + 1        # v width incl. ones column
P = 128
KT = D // P        # 8 contraction tiles over D
SQ = 512           # query/N chunk
NSQ = S // SQ      # 4
NST = S // P       # 16 seq tiles
SCALE = 1.0 / np.sqrt(HD)


def emit_kernel(nc, tc, ap):
    """Emit the per-core program. `ap` is a dict of DRAM APs."""
    with (
        tc.tile_pool(name="const", bufs=1) as cp,
        tc.tile_pool(name="xw", bufs=1) as xw,
        tc.tile_pool(name="act", bufs=1) as acts,
        tc.tile_pool(name="ex", bufs=16) as exp_pool,
        tc.tile_pool(name="dh", bufs=4) as dh_pool,
        tc.tile_pool(name="rc", bufs=2) as rc_pool,
        tc.tile_pool(name="osb", bufs=3) as osb,
        tc.tile_pool(name="psA", bufs=2, space="PSUM") as psA,
        tc.tile_pool(name="psB", bufs=2, space="PSUM") as psB,
        tc.tile_pool(name="psC", bufs=2, space="PSUM") as psC,
    ):
        # ---- weight/x DMAs on the sync (HWDGE) ring, in consumption order.
        # All sources are host-prepared in the exact SBUF layout (contiguous
        # per-partition blocks) so the DMAs run at line rate. x comes in 4
        # seq-quarters so QKV chunk c is gated only on quarter c.
        # the first two gating transfers (wq, x quarter 0) go out on the
        # scalar engine's HWDGE ring, whose NEFF preamble finishes ~2us
        # before the sync ring's; the rest stream on sync in order
        xts = xw.tile([P, KT, S], F16, name="xts", tag="xts")
        wq = xw.tile([P, KT, CH], F16, name="wq", tag="wq")
        nc.scalar.dma_start(wq, ap["wq"])
        nc.scalar.dma_start(xts[:, :, 0:SQ], ap["xT"][:, 0])
        wk = xw.tile([P, KT, CH], F16, name="wk", tag="wk")
        nc.sync.dma_start(wk, ap["wk"])
        bq = cp.tile([P, 2], F32, name="bq", tag="bq")
        nc.sync.dma_start(bq, ap["bq"])
        bk = cp.tile([P, 2], F32, name="bk", tag="bk")
        nc.sync.dma_start(bk, ap["bk"])
        wv = xw.tile([P, KT, HPC * VW], F16, name="wv", tag="wv")
        nc.sync.dma_start(wv, ap["wv"])
        for c in range(1, NSQ):
            nc.sync.dma_start(xts[:, :, c * SQ:(c + 1) * SQ], ap["xT"][:, c])
        wp = xw.tile([P, 2, D], F16, name="wp", tag="wp")
        nc.sync.dma_start(wp, ap["wp"])

        # warmup scratch zeroed on gpsimd BEFORE its const DMAs queue up
        # (and not on the vector engine, whose preamble lands ~5us in)
        wsrc = cp.tile([P, SQ], F16, name="wsrc", tag="wsrc")
        nc.gpsimd.memset(wsrc, 0.0)

        # small consts on the gpsimd SWDGE ring, in parallel with the above
        ones1 = cp.tile([1, P], F16, name="ones1", tag="ones1")
        nc.gpsimd.dma_start(ones1, ap["ones1"])
        bv = cp.tile([1, HPC * VW], F16, name="bv", tag="bv")
        nc.gpsimd.dma_start(bv, ap["bv"])
        tri = cp.tile([P, P], F16, name="tri", tag="tri")
        nc.gpsimd.dma_start(tri, ap["tri"])

        # ---- PE warmup: dense dummy matmuls while input DMAs stream in.
        # The PE clock-gate (HAM) unthrottles 1.2->2.4 GHz only after ~3.4us
        # of sustained matmul activity; burn that in on scratch data.
        # 26 matmuls cover the ~7us sync-queue preamble + the first weight/x
        # transfers at the cold (1.2 GHz) rate, so the PE is warm and fed
        # when the first real matmul's inputs land.
        wps = psB.tile([P, SQ], F32, name="wps", tag="acc")
        for i in range(16):
            nc.tensor.matmul(
                wps, wsrc[:, 0:P], wsrc, start=(i == 0), stop=(i == 15),
            )

        xts_k = [xts[:, k, :] for k in range(KT)]
        wq_t = [wq[:, k, :] for k in range(KT)]
        wk_t = [wk[:, k, :] for k in range(KT)]
        wv_t = [wv[:, k, :] for k in range(KT)]
        wp_t = [wp[:, k, :] for k in range(2)]

        # ---- activations living across phases ----
        qT = acts.tile([P, 2, S], F16, name="qT", tag="qT")
        kTt = acts.tile([P, 2, S], F16, name="kT", tag="kT")
        vv = acts.tile([P, NST, HPC * VW], F16, name="vv", tag="vv")
        outT = [acts.tile([P, S], F16, name=f"oT{i}", tag=f"oT{i}") for i in range(2)]

        def qk_pair(c, dst, wt, bias, pool):
            """Both i-halves of a q or k projection chunk, as interleaved
            chains into two PSUM regions (psC: one 2-bank tile; psA: two
            1-bank tiles) so consecutive accumulates hit different banks."""
            if pool is psC:
                ps2 = psC.tile([P, 2, SQ], F32, name="sc2", tag="sc")
                pss = [ps2[:, 0, :], ps2[:, 1, :]]
            else:
                pss = [psA.tile([P, SQ], F32, name="ps", tag="ps")
                       for _ in range(2)]
            for k in range(KT):
                for i in range(2):
                    nc.tensor.matmul(
                        pss[i],
                        wt[k][:, i * P:(i + 1) * P],
                        xts_k[k][:, c * SQ:(c + 1) * SQ],
                        start=(k == 0),
                        stop=(k == KT - 1),
                    )
            with nc.allow_low_precision(reason="fp16 matmul inputs"):
                if pool is psC:
                    nc.vector.tensor_add(
                        dst[:, :, c * SQ:(c + 1) * SQ], ps2,
                        bias[:, :, None].broadcast_to([P, 2, SQ]),
                    )
                else:
                    for i in range(2):
                        nc.vector.tensor_scalar_add(
                            dst[:, i, c * SQ:(c + 1) * SQ], pss[i],
                            bias[:, i:i + 1],
                        )

        def v_pair(st0, pool):
            """Two v seq-tiles as interleaved chains (natural layout +
            interleaved ones cols; the trailing ones matmul adds v-bias and
            the denominator ones column)."""
            if pool is psC:
                ps2 = psC.tile([P, 2, SQ], F32, name="sc2", tag="sc")
                pss = [ps2[:, 0, 0:HPC * VW], ps2[:, 1, 0:HPC * VW]]
            else:
                pss = [psA.tile([P, SQ], F32, name="psv", tag="ps")[:, 0:HPC * VW]
                       for _ in range(2)]
            for k in range(KT):
                for ci in range(2):
                    nc.tensor.matmul(
                        pss[ci],
                        xts_k[k][:, (st0 + ci) * P:(st0 + ci + 1) * P],
                        wv_t[k],
                        start=(k == 0),
                        stop=False,
                    )
            for ci in range(2):
                nc.tensor.matmul(pss[ci], ones1, bv, start=False, stop=True)
            with nc.allow_low_precision(reason="fp16 matmul inputs"):
                for ci in range(2):
                    nc.vector.tensor_copy(vv[:, st0 + ci, :], pss[ci])

        def qkv_groups(c, pool):
            yield lambda: qk_pair(c, qT, wq_t, bq, pool)
            yield lambda: qk_pair(c, kTt, wk_t, bk, pool)
            yield lambda: v_pair(4 * c, pool)
            yield lambda: v_pair(4 * c + 2, pool)

        def attention_pair(i, c, fillers=(), tail=False):
            """Heads 2i (kT/qT partition rows 0:64) and 2i+1 (rows 64:128).

            Both heads' scores for a key tile land in one 2-bank PSUM tile so
            a single exp instruction covers them (halves ScalarE instruction
            count). All scores are emitted before all attnv matmuls: the PE
            stream is in-order, so ScalarE's exps pipeline behind the score
            stream. The PE stalls on exp pacing both in the scores loop (sc
            PSUM rotation) and the attnv loop, so fillers pop on a schedule
            spread across BOTH loops. Diagonal blocks get their masked
            triangle zeroed by a GpSimd multiply with the tri mask."""
            nkt = 4 * (c + 1)
            accs = [psB.tile([VW, SQ], F32, name="acc", tag="acc")
                    for _ in range(2)]
            fillers = list(fillers)
            nf = len(fillers)
            steps = 2 * nkt

            def pop_due(step):
                while fillers and len(fillers) > nf * (steps - 1 - step) // steps:
                    fillers.pop(0)()

            exs = []
            for kt in range(nkt):
                colo = max(0, kt * P - c * SQ)
                diag = colo > 0 or kt * P == c * SQ
                sc2 = psC.tile([P, 2, SQ], F32, name="sc2", tag="sc")
                for j in range(2):
                    ro = j * 64
                    nc.tensor.matmul(
                        sc2[:, j, colo:SQ],
                        kTt[ro:ro + 64, i, kt * P:(kt + 1) * P],
                        qT[ro:ro + 64, i, c * SQ + colo:(c + 1) * SQ],
                        start=True,
                        stop=True,
                    )
                ex2 = exp_pool.tile([P, 2, SQ], F16, name="ex2", tag="ex")
                nc.scalar.activation(
                    ex2[:, :, colo:SQ], sc2[:, :, colo:SQ], AFT.Exp, scale=SCALE,
                )
                if diag:
                    # zero the masked triangle of the diagonal block on the
                    # (otherwise idle) GpSimd engine
                    nc.gpsimd.tensor_mul(
                        ex2[:, :, colo:colo + P],
                        ex2[:, :, colo:colo + P],
                        tri[:, None, :].broadcast_to([P, 2, P]),
                    )
                exs.append((ex2, kt, colo))
                # the scores loop is itself exp-paced (the sc PSUM slots
                # recycle only as ScalarE drains), so fillers pop here too
                if kt >= 2:
                    pop_due(kt - 2)
            for ex2, kt, colo in exs:
                for j in range(2):
                    h = 2 * i + j
                    nc.tensor.matmul(
                        accs[j][:, colo:SQ],
                        vv[:, kt, h * VW:(h + 1) * VW],
                        ex2[:, j, colo:SQ],
                        start=(kt == 0),
                        stop=(kt == nkt - 1),
                    )
                # dense PE filler between exp-paced attnv groups
                pop_due(nkt - 2 + kt)
            # the tiny denominator copies go first so the norm chain
            # (db matmuls -> reciprocal) starts as early as possible
            dns = []
            for j in range(2):
                dn = dh_pool.tile([1, SQ], F16, name="dn", tag="dn")
                with nc.allow_low_precision(reason="fp16 matmul inputs"):
                    nc.vector.tensor_copy(dn, accs[j][64:65, :])
                dns.append(dn)
            for j in range(2):
                # at the tail ScalarE is idle: move the big outT copies there
                # so the DVE queue reaches the reciprocal/normalize sooner
                with nc.allow_low_precision(reason="fp16 matmul inputs"):
                    dst = outT[i][j * 64:j * 64 + 64, c * SQ:(c + 1) * SQ]
                    if tail:
                        nc.scalar.copy(dst, accs[j][0:64, :])
                    else:
                        nc.vector.tensor_copy(dst, accs[j][0:64, :])
            return dns

        def norm_pair(c, i, dns, pool=None):
            # outT *= 1/denominator: broadcast denoms via K=1 matmuls (the
            # two M=64 matmuls col-tile concurrently via base_partition),
            # one 128-lane fast reciprocal, one fp16 multiply
            db = (pool or psA).tile([P, SQ], F32, name="db",
                                    tag="acc" if pool is psB else "ps")
            nc.tensor.matmul(
                db[0:64, :], ones1[:, 0:64], dns[0],
                start=True, stop=True,
            )
            nc.tensor.matmul(
                db[64:P, :], ones1[:, 0:64], dns[1],
                start=True, stop=True,
            )
            rc32 = rc_pool.tile([P, SQ], F32, name="rc32", tag="rc32")
            nc.vector.reciprocal_approx_fast(rc32, db)
            with nc.allow_low_precision(reason="fp16 matmul inputs"):
                nc.vector.tensor_mul(
                    outT[i][:, c * SQ:(c + 1) * SQ],
                    outT[i][:, c * SQ:(c + 1) * SQ],
                    rc32,
                )

        def proj_mtile(m, tail=False):
            # one m-tile of the projection: [128 seq, 1024 outdims]; the two
            # nch contraction chains interleave across two PSUM tiles, and
            # both halves land in one ob tile -> a single 256KB out DMA
            ob = osb.tile([P, 2 * SQ], F16, name="ob", tag="ob")
            pss = [psA.tile([P, SQ], F32, name="ps", tag="ps")
                   for _ in range(2)]
            for kk in range(2):
                for nch in range(2):
                    nc.tensor.matmul(
                        pss[nch],
                        outT[kk][:, m * P:(m + 1) * P],
                        wp_t[kk][:, nch * SQ:(nch + 1) * SQ],
                        start=(kk == 0),
                        stop=(kk == 1),
                    )
            for nch in range(2):
                with nc.allow_low_precision(reason="partial sums; host sums fp32"):
                    # at the tail ScalarE is idle: split the two copies
                    # across ScalarE and DVE so they run concurrently
                    if tail and nch == 0:
                        nc.scalar.copy(ob[:, 0:SQ], pss[0])
                    else:
                        nc.vector.tensor_copy(
                            ob[:, nch * SQ:(nch + 1) * SQ], pss[nch])
                if tail:
                    # half-tile DMAs off alternating rings so the last
                    # transfer is small and starts as early as possible
                    eng = nc.gpsimd if nch == 0 else nc.sync
                    eng.dma_start(
                        ap["out"][m * P:(m + 1) * P, nch * SQ:(nch + 1) * SQ],
                        ob[:, nch * SQ:(nch + 1) * SQ])
            if not tail:
                nc.sync.dma_start(ap["out"][m * P:(m + 1) * P, :], ob)

        # ---- chunk-pipelined main body ----
        # chunk 0 QKV upfront through the score-PSUM pool (scores aren't
        # running yet), with warmup matmuls between the early groups to keep
        # the PE clock-gate busy while the x quarters stream in
        for gi, g in enumerate(qkv_groups(0, psC)):
            g()
            if gi < 2:
                for i in range(2):
                    nc.tensor.matmul(
                        wps, wsrc[:, 0:P], wsrc,
                        start=(i == 0), stop=(i == 1),
                    )
        # attention(c) runs against qkv chunks emitted one chunk ahead.
        # Filler slots between the exp-paced attnv groups carry, in order:
        # the previous pair's norm, the next chunk's qkv groups, and proj
        # m-tiles of chunks whose norms are complete. Chunk t's proj
        # (m=4t..4t+3) becomes eligible after norm(t,1).
        filler_plan = {
            (0, 0): [],
            (0, 1): ["n00"],
            (1, 0): ["n01"],
            (1, 1): ["n10", "m0"],
            (2, 0): ["n11", "m1", "m2"],
            (2, 1): ["n20", "m3", "m4", "m5"],
            (3, 0): ["n21", "m6", "m7", "m8", "m9", "m10", "m11"],
            (3, 1): ["n30"],
        }
        norms = {}          # (c, i) -> dns, filled as pairs complete

        def make_filler(tok):
            if tok.startswith("n"):
                c, i = int(tok[1]), int(tok[2])
                return lambda: norm_pair(c, i, norms[(c, i)])
            m = int(tok[1:])
            return lambda: proj_mtile(m)

        for c in range(NSQ):
            nxt = list(qkv_groups(c + 1, psA)) if c + 1 < NSQ else []
            for i in range(2):
                fillers = [make_filler(t) for t in filler_plan[(c, i)]]
                # next chunk's qkv interleaves after the norm filler
                fillers[1:1] = nxt[2 * i:2 * i + 2]
                norms[(c, i)] = attention_pair(
                    i, c, fillers, tail=(c == NSQ - 1 and i == 1))
        # tail: the last norm's broadcast goes through psB (psA would make
        # the first proj tile wait on the reciprocal), then chunk 3's proj
        norm_pair(NSQ - 1, 1, norms[(NSQ - 1, 1)], pool=psB)
        for m in range(12, 16):
            proj_mtile(m, tail=True)


def build_program():
    nc = bacc.Bacc("TRN2", target_bir_lowering=False, debug=False,
                   num_devices=NCORES)
    ap = {}
    for name, shape, dt in (
        ("xT", [P, NSQ, KT, SQ], F16), ("wq", [P, KT, CH], F16),
        ("wk", [P, KT, CH], F16), ("wv", [P, KT, HPC * VW], F16),
        ("bq", [P, 2], F32), ("bk", [P, 2], F32),
        ("bv", [1, HPC * VW], F16), ("wp", [P, 2, D], F16),
        ("tri", [P, P], F16), ("ones1", [1, P], F16),
    ):
        ap[name] = nc.dram_tensor(name, shape, dt, kind="ExternalInput").ap()
    ap["out"] = nc.dram_tensor("out", [S, D], F16, kind="ExternalOutput").ap()

    with tile.TileContext(nc) as tc:
        emit_kernel(nc, tc, ap)
    nc.compile()
    return nc


def make_core_inputs(hidden_states, w_attn, b_attn, w_proj):
    """Host-side sharding: per-core input dicts (core = batch*4 + head_group)."""
    f16, f32 = np.float16, np.float32
    x = np.asarray(hidden_states, f32)
    w_attn = np.asarray(w_attn, f32)
    b_attn = np.asarray(b_attn, f32)
    w_proj = np.asarray(w_proj, f32)

    tri = (np.arange(P)[:, None] <= np.arange(P)[None, :]).astype(f16)
    ones_row = np.ones((1, P), f16)

    def kmaj(w):
        # [D=(KT P), C] -> [P, KT, C] contiguous (the SBUF tile layout)
        return np.ascontiguousarray(
            w.reshape(KT, P, -1).transpose(1, 0, 2)).astype(f16)

    # x.T [(KT P), (NSQ SQ)] -> [P, NSQ, KT, SQ] contiguous seq-quarters
    xTs = [np.ascontiguousarray(
        x[b].T.reshape(KT, P, NSQ, SQ).transpose(1, 2, 0, 3)).astype(f16)
        for b in range(B)]

    in_maps = []
    for core in range(NCORES):
        b, g = core // HPC, core % HPC
        wq = kmaj(w_attn[:, g * CH:(g + 1) * CH])
        wk = kmaj(w_attn[:, D + g * CH:D + (g + 1) * CH])
        wv = np.zeros((D, HPC * VW), np.float32)
        bv = np.zeros((1, HPC * VW), f16)
        for h in range(HPC):
            src = 2 * D + (g * HPC + h) * HD
            wv[:, h * VW:h * VW + HD] = w_attn[:, src:src + HD]
            bv[0, h * VW:h * VW + HD] = b_attn[src:src + HD]
            bv[0, h * VW + HD] = 1.0
        wv = kmaj(wv)
        bq = np.ascontiguousarray(
            b_attn[g * CH:(g + 1) * CH].reshape(2, P).T)
        bk = np.ascontiguousarray(
            b_attn[D + g * CH:D + (g + 1) * CH].reshape(2, P).T)
        wp = np.ascontiguousarray(
            w_proj[g * CH:(g + 1) * CH, :].reshape(2, P, D)
            .transpose(1, 0, 2)).astype(f16)
        in_maps.append({
            "xT": xTs[b], "wq": wq, "wk": wk, "wv": wv,
            "bq": bq, "bk": bk, "bv": bv, "wp": wp,
            "tri": tri, "ones1": ones_row,
        })
    return in_maps


_PROGRAM = None


def kernel(hidden_states, w_attn, b_attn, w_proj, b_proj):
    global _PROGRAM
    if _PROGRAM is None:
        _PROGRAM = build_program()
    in_maps = make_core_inputs(hidden_states, w_attn, b_attn, w_proj)
    res = run_bass_kernel_spmd(_PROGRAM, in_maps, core_ids=list(range(NCORES)))
    out = np.zeros((B, S, D), np.float32)
    for core in range(NCORES):
        out[core // HPC] += res.results[core]["out"].astype(np.float32)
    out += np.asarray(b_proj, np.float32)
    return out


# revision 22
# speedup vs baseline: 1.2346x; 1.2346x over previous
"""Fused causal multi-head attention block on 8 Trainium2 NeuronCores.

Problem (GPT-2 style attention, B=2, S=2048, D=1024, H=16, hd=64):
    qkv = x @ w_attn + b_attn ; split q,k,v ; per-head causal softmax(q k^T / 8) v
    out = attn_out @ w_proj + b_proj

Sharding: data parallel on batch (2) x tensor parallel on heads (4 groups of 4
heads). Core c -> batch c//4, head group c%4. Each core computes a partial
[S, D] output (its heads' slice of w_proj rows); host sums the 4 partials per
batch and adds b_proj.

Per-core kernel layout tricks:
- scores are computed TRANSPOSED (scoresT[key, query]) so the softmax
  denominator falls out of the attn@v matmul by appending a ones-column to v:
  [v | 1]^T @ exp(scoresT) yields the unnormalized output and the per-query
  denominator in one PSUM accumulation.
- matmul inputs are fp16 (full PE rate + fast weight loads); all accumulation
  is fp32 in PSUM. exp(s/8) is in [0, ~13], well inside fp16 range.
- back-to-back matmuls accumulating into the SAME PSUM region serialize on
  the array drain (~175ns each). All contraction chains (qkv, proj) are
  emitted as interleaved PAIRS of chains targeting two PSUM regions, so one
  chain's fill overlaps the other's drain.
- causal masking: fully-masked blocks are skipped via restricted matmul
  widths; diagonal blocks get their masked triangle zeroed on the (otherwise
  idle) GpSimd engine, keeping both the PE and the vector engine out of the
  score->attnv chain.
- x is DMA'd in 4 seq-quarters so QKV chunk c only waits for quarter c;
  weight DMAs are ordered ahead of the x quarter they gate. Small consts go
  out on the gpsimd SWDGE ring in parallel with the sync ring.
- emission is chunk-pipelined: QKV chunk c+1, proj of completed chunks, and
  the previous pair's normalization are spread as PE filler between the
  exp-paced attnv groups, so the PE has dense matmul work while ScalarE runs
  exp. Two fillers are popped before the first attnv group to cover the
  ScalarE backlog at pair transitions.
- the two per-pair score matmuls (K=64 at partition bases 0/64) row-tile into
  the PE array concurrently; the two norm broadcast matmuls (M=64 at output
  bases 0/64) col-tile concurrently.
"""

import sys

sys.path.insert(0, "/opt/trn_rl_repo")

import numpy as np

import concourse.bass as bass
import concourse.mybir as mybir
import concourse.tile as tile
from concourse import bacc
from concourse.bass_utils import run_bass_kernel_spmd

F32 = mybir.dt.float32
F16 = mybir.dt.float16
AFT = mybir.ActivationFunctionType

B, S, D, H, HD = 2, 2048, 1024, 16, 64
NCORES = 8
HPC = 4            # heads per core
CH = HPC * HD      # 256 channels per core
VW = HD + 1        # v width incl. ones column
P = 128
KT = D // P        # 8 contraction tiles over D
SQ = 512           # query/N chunk
NSQ = S // SQ      # 4
NST = S // P       # 16 seq tiles
SCALE = 1.0 / np.sqrt(HD)


def emit_kernel(nc, tc, ap):
    """Emit the per-core program. `ap` is a dict of DRAM APs."""
    with (
        tc.tile_pool(name="const", bufs=1) as cp,
        tc.tile_pool(name="xw", bufs=1) as xw,
        tc.tile_pool(name="act", bufs=1) as acts,
        tc.tile_pool(name="ex", bufs=16) as exp_pool,
        tc.tile_pool(name="dh", bufs=4) as dh_pool,
        tc.tile_pool(name="rc", bufs=2) as rc_pool,
        tc.tile_pool(name="osb", bufs=3) as osb,
        tc.tile_pool(name="psA", bufs=2, space="PSUM") as psA,
        tc.tile_pool(name="psB", bufs=2, space="PSUM") as psB,
        tc.tile_pool(name="psC", bufs=2, space="PSUM") as psC,
    ):
        # ---- weight/x DMAs on the sync (HWDGE) ring, in consumption order.
        # All sources are host-prepared in the exact SBUF layout (contiguous
        # per-partition blocks) so the DMAs run at line rate. x comes in 4
        # seq-quarters so QKV chunk c is gated only on quarter c.
        # everything streams on the single sync HWDGE ring in consumption
        # order: the SDMA engines round-robin across rings with no priority,
        # so a second ring would steal HBM bandwidth from the gating
        # transfers at the front of this one
        wq = xw.tile([P, KT, CH], F16, name="wq", tag="wq")
        nc.sync.dma_start(wq, ap["wq"])
        wk = xw.tile([P, KT, CH], F16, name="wk", tag="wk")
        nc.sync.dma_start(wk, ap["wk"])
        bq = cp.tile([P, 2], F32, name="bq", tag="bq")
        nc.sync.dma_start(bq, ap["bq"])
        bk = cp.tile([P, 2], F32, name="bk", tag="bk")
        nc.sync.dma_start(bk, ap["bk"])
        xts = xw.tile([P, KT, S], F16, name="xts", tag="xts")
        for c in range(NSQ):
            nc.sync.dma_start(xts[:, :, c * SQ:(c + 1) * SQ], ap["xT"][:, c])
            if c == 0:
                wv = xw.tile([P, KT, HPC * VW], F16, name="wv", tag="wv")
                nc.sync.dma_start(wv, ap["wv"])
        wp = xw.tile([P, 2, D], F16, name="wp", tag="wp")
        nc.sync.dma_start(wp, ap["wp"])

        # warmup scratch zeroed on gpsimd BEFORE its const DMAs queue up
        # (and not on the vector engine, whose preamble lands ~5us in)
        wsrc = cp.tile([P, SQ], F16, name="wsrc", tag="wsrc")
        nc.gpsimd.memset(wsrc, 0.0)

        # small consts on the gpsimd SWDGE ring, in parallel with the above
        ones1 = cp.tile([1, P], F16, name="ones1", tag="ones1")
        nc.gpsimd.dma_start(ones1, ap["ones1"])
        bv = cp.tile([1, HPC * VW], F16, name="bv", tag="bv")
        nc.gpsimd.dma_start(bv, ap["bv"])
        tri = cp.tile([P, P], F16, name="tri", tag="tri")
        nc.gpsimd.dma_start(tri, ap["tri"])

        # ---- PE warmup: dense dummy matmuls while input DMAs stream in.
        # The PE clock-gate (HAM) unthrottles 1.2->2.4 GHz only after ~3.4us
        # of sustained matmul activity; burn that in on scratch data.
        # 26 matmuls cover the ~7us sync-queue preamble + the first weight/x
        # transfers at the cold (1.2 GHz) rate, so the PE is warm and fed
        # when the first real matmul's inputs land.
        wps = psB.tile([P, SQ], F32, name="wps", tag="acc")
        for i in range(26):
            nc.tensor.matmul(
                wps, wsrc[:, 0:P], wsrc, start=(i == 0), stop=(i == 25),
            )

        xts_k = [xts[:, k, :] for k in range(KT)]
        wq_t = [wq[:, k, :] for k in range(KT)]
        wk_t = [wk[:, k, :] for k in range(KT)]
        wv_t = [wv[:, k, :] for k in range(KT)]
        wp_t = [wp[:, k, :] for k in range(2)]

        # ---- activations living across phases ----
        qT = acts.tile([P, 2, S], F16, name="qT", tag="qT")
        kTt = acts.tile([P, 2, S], F16, name="kT", tag="kT")
        vv = acts.tile([P, NST, HPC * VW], F16, name="vv", tag="vv")
        outT = [acts.tile([P, S], F16, name=f"oT{i}", tag=f"oT{i}") for i in range(2)]

        def qk_pair(c, dst, wt, bias, pool):
            """Both i-halves of a q or k projection chunk, as interleaved
            chains into two PSUM regions (psC: one 2-bank tile; psA: two
            1-bank tiles) so consecutive accumulates hit different banks."""
            if pool is psC:
                ps2 = psC.tile([P, 2, SQ], F32, name="sc2", tag="sc")
                pss = [ps2[:, 0, :], ps2[:, 1, :]]
            else:
                pss = [psA.tile([P, SQ], F32, name="ps", tag="ps")
                       for _ in range(2)]
            for k in range(KT):
                for i in range(2):
                    nc.tensor.matmul(
                        pss[i],
                        wt[k][:, i * P:(i + 1) * P],
                        xts_k[k][:, c * SQ:(c + 1) * SQ],
                        start=(k == 0),
                        stop=(k == KT - 1),
                    )
            with nc.allow_low_precision(reason="fp16 matmul inputs"):
                if pool is psC:
                    nc.vector.tensor_add(
                        dst[:, :, c * SQ:(c + 1) * SQ], ps2,
                        bias[:, :, None].broadcast_to([P, 2, SQ]),
                    )
                else:
                    for i in range(2):
                        nc.vector.tensor_scalar_add(
                            dst[:, i, c * SQ:(c + 1) * SQ], pss[i],
                            bias[:, i:i + 1],
                        )

        def v_pair(st0, pool):
            """Two v seq-tiles as interleaved chains (natural layout +
            interleaved ones cols; the trailing ones matmul adds v-bias and
            the denominator ones column)."""
            if pool is psC:
                ps2 = psC.tile([P, 2, SQ], F32, name="sc2", tag="sc")
                pss = [ps2[:, 0, 0:HPC * VW], ps2[:, 1, 0:HPC * VW]]
            else:
                pss = [psA.tile([P, SQ], F32, name="psv", tag="ps")[:, 0:HPC * VW]
                       for _ in range(2)]
            for k in range(KT):
                for ci in range(2):
                    nc.tensor.matmul(
                        pss[ci],
                        xts_k[k][:, (st0 + ci) * P:(st0 + ci + 1) * P],
                        wv_t[k],
                        start=(k == 0),
                        stop=False,
                    )
            for ci in range(2):
                nc.tensor.matmul(pss[ci], ones1, bv, start=False, stop=True)
            with nc.allow_low_precision(reason="fp16 matmul inputs"):
                for ci in range(2):
                    nc.vector.tensor_copy(vv[:, st0 + ci, :], pss[ci])

        def qkv_groups(c, pool):
            yield lambda: qk_pair(c, qT, wq_t, bq, pool)
            yield lambda: qk_pair(c, kTt, wk_t, bk, pool)
            yield lambda: v_pair(4 * c, pool)
            yield lambda: v_pair(4 * c + 2, pool)

        def attention_pair(i, c, fillers=(), tail=False):
            """Heads 2i (kT/qT partition rows 0:64) and 2i+1 (rows 64:128).

            Both heads' scores for a key tile land in one 2-bank PSUM tile so
            a single exp instruction covers them (halves ScalarE instruction
            count). All scores are emitted before all attnv matmuls: the PE
            stream is in-order, so ScalarE's exps pipeline behind the score
            stream. The PE stalls on exp pacing both in the scores loop (sc
            PSUM rotation) and the attnv loop, so fillers pop on a schedule
            spread across BOTH loops. Diagonal blocks get their masked
            triangle zeroed by a GpSimd multiply with the tri mask."""
            nkt = 4 * (c + 1)
            accs = [psB.tile([VW, SQ], F32, name="acc", tag="acc")
                    for _ in range(2)]
            fillers = list(fillers)
            nf = len(fillers)
            steps = 2 * nkt

            def pop_due(step):
                while fillers and len(fillers) > nf * (steps - 1 - step) // steps:
                    fillers.pop(0)()

            exs = []
            for kt in range(nkt):
                colo = max(0, kt * P - c * SQ)
                diag = colo > 0 or kt * P == c * SQ
                sc2 = psC.tile([P, 2, SQ], F32, name="sc2", tag="sc")
                for j in range(2):
                    ro = j * 64
                    nc.tensor.matmul(
                        sc2[:, j, colo:SQ],
                        kTt[ro:ro + 64, i, kt * P:(kt + 1) * P],
                        qT[ro:ro + 64, i, c * SQ + colo:(c + 1) * SQ],
                        start=True,
                        stop=True,
                    )
                ex2 = exp_pool.tile([P, 2, SQ], F16, name="ex2", tag="ex")
                nc.scalar.activation(
                    ex2[:, :, colo:SQ], sc2[:, :, colo:SQ], AFT.Exp, scale=SCALE,
                )
                if diag:
                    # zero the masked triangle of the diagonal block on the
                    # (otherwise idle) GpSimd engine
                    nc.gpsimd.tensor_mul(
                        ex2[:, :, colo:colo + P],
                        ex2[:, :, colo:colo + P],
                        tri[:, None, :].broadcast_to([P, 2, P]),
                    )
                exs.append((ex2, kt, colo))
                # the scores loop is itself exp-paced (the sc PSUM slots
                # recycle only as ScalarE drains), so fillers pop here too
                if kt >= 2:
                    pop_due(kt - 2)
            for ex2, kt, colo in exs:
                for j in range(2):
                    h = 2 * i + j
                    nc.tensor.matmul(
                        accs[j][:, colo:SQ],
                        vv[:, kt, h * VW:(h + 1) * VW],
                        ex2[:, j, colo:SQ],
                        start=(kt == 0),
                        stop=(kt == nkt - 1),
                    )
                # dense PE filler between exp-paced attnv groups
                pop_due(nkt - 2 + kt)
            # the tiny denominator copies go first so the norm chain
            # (db matmuls -> reciprocal) starts as early as possible
            dns = []
            for j in range(2):
                dn = dh_pool.tile([1, SQ], F16, name="dn", tag="dn")
                with nc.allow_low_precision(reason="fp16 matmul inputs"):
                    nc.vector.tensor_copy(dn, accs[j][64:65, :])
                dns.append(dn)
            for j in range(2):
                # at the tail ScalarE is idle: move the big outT copies there
                # so the DVE queue reaches the reciprocal/normalize sooner
                with nc.allow_low_precision(reason="fp16 matmul inputs"):
                    dst = outT[i][j * 64:j * 64 + 64, c * SQ:(c + 1) * SQ]
                    if tail:
                        nc.scalar.copy(dst, accs[j][0:64, :])
                    else:
                        nc.vector.tensor_copy(dst, accs[j][0:64, :])
            return dns

        def norm_pair(c, i, dns, pool=None):
            # outT *= 1/denominator: broadcast denoms via K=1 matmuls (the
            # two M=64 matmuls col-tile concurrently via base_partition),
            # one 128-lane fast reciprocal, one fp16 multiply
            db = (pool or psA).tile([P, SQ], F32, name="db",
                                    tag="acc" if pool is psB else "ps")
            nc.tensor.matmul(
                db[0:64, :], ones1[:, 0:64], dns[0],
                start=True, stop=True,
            )
            nc.tensor.matmul(
                db[64:P, :], ones1[:, 0:64], dns[1],
                start=True, stop=True,
            )
            rc32 = rc_pool.tile([P, SQ], F32, name="rc32", tag="rc32")
            nc.vector.reciprocal_approx_fast(rc32, db)
            with nc.allow_low_precision(reason="fp16 matmul inputs"):
                nc.vector.tensor_mul(
                    outT[i][:, c * SQ:(c + 1) * SQ],
                    outT[i][:, c * SQ:(c + 1) * SQ],
                    rc32,
                )

        def proj_mtile(m, tail=False):
            # one m-tile of the projection: [128 seq, 1024 outdims]; the two
            # nch contraction chains interleave across two PSUM tiles, and
            # both halves land in one ob tile -> a single 256KB out DMA
            ob = osb.tile([P, 2 * SQ], F16, name="ob", tag="ob")
            pss = [psA.tile([P, SQ], F32, name="ps", tag="ps")
                   for _ in range(2)]
            for kk in range(2):
                for nch in range(2):
                    nc.tensor.matmul(
                        pss[nch],
                        outT[kk][:, m * P:(m + 1) * P],
                        wp_t[kk][:, nch * SQ:(nch + 1) * SQ],
                        start=(kk == 0),
                        stop=(kk == 1),
                    )
            for nch in range(2):
                with nc.allow_low_precision(reason="partial sums; host sums fp32"):
                    # at the tail ScalarE is idle: split the two copies
                    # across ScalarE and DVE so they run concurrently
                    if tail and nch == 0:
                        nc.scalar.copy(ob[:, 0:SQ], pss[0])
                    else:
                        nc.vector.tensor_copy(
                            ob[:, nch * SQ:(nch + 1) * SQ], pss[nch])
                if tail:
                    # half-tile DMAs off alternating rings so the last
                    # transfer is small and starts as early as possible
                    eng = nc.gpsimd if nch == 0 else nc.sync
                    eng.dma_start(
                        ap["out"][m * P:(m + 1) * P, nch * SQ:(nch + 1) * SQ],
                        ob[:, nch * SQ:(nch + 1) * SQ])
            if not tail:
                nc.sync.dma_start(ap["out"][m * P:(m + 1) * P, :], ob)

        # ---- chunk-pipelined main body ----
        # chunk 0 QKV upfront through the score-PSUM pool (scores aren't
        # running yet), with warmup matmuls between the early groups to keep
        # the PE clock-gate busy while the x quarters stream in
        for gi, g in enumerate(qkv_groups(0, psC)):
            g()
            if gi < 2:
                for i in range(2):
                    nc.tensor.matmul(
                        wps, wsrc[:, 0:P], wsrc,
                        start=(i == 0), stop=(i == 1),
                    )
        # attention(c) runs against qkv chunks emitted one chunk ahead.
        # Filler slots between the exp-paced attnv groups carry, in order:
        # the previous pair's norm, the next chunk's qkv groups, and proj
        # m-tiles of chunks whose norms are complete. Chunk t's proj
        # (m=4t..4t+3) becomes eligible after norm(t,1).
        filler_plan = {
            (0, 0): [],
            (0, 1): ["n00"],
            (1, 0): ["n01"],
            (1, 1): ["n10", "m0"],
            (2, 0): ["n11", "m1", "m2"],
            (2, 1): ["n20", "m3", "m4", "m5"],
            (3, 0): ["n21", "m6", "m7", "m8", "m9", "m10", "m11"],
            (3, 1): ["n30"],
        }
        norms = {}          # (c, i) -> dns, filled as pairs complete

        def make_filler(tok):
            if tok.startswith("n"):
                c, i = int(tok[1]), int(tok[2])
                return lambda: norm_pair(c, i, norms[(c, i)])
            m = int(tok[1:])
            return lambda: proj_mtile(m)

        for c in range(NSQ):
            nxt = list(qkv_groups(c + 1, psA)) if c + 1 < NSQ else []
            for i in range(2):
                fillers = [make_filler(t) for t in filler_plan[(c, i)]]
                # next chunk's qkv interleaves after the norm filler
                fillers[1:1] = nxt[2 * i:2 * i + 2]
                norms[(c, i)] = attention_pair(
                    i, c, fillers, tail=(c == NSQ - 1 and i == 1))
        # tail: the last norm's broadcast goes through psB (psA would make
        # the first proj tile wait on the reciprocal), then chunk 3's proj
        norm_pair(NSQ - 1, 1, norms[(NSQ - 1, 1)], pool=psB)
        for m in range(12, 16):
            proj_mtile(m, tail=True)


def build_program():
    nc = bacc.Bacc("TRN2", target_bir_lowering=False, debug=False,
                   num_devices=NCORES)
    ap = {}
    for name, shape, dt in (
        ("xT", [P, NSQ, KT, SQ], F16), ("wq", [P, KT, CH], F16),
        ("wk", [P, KT, CH], F16), ("wv", [P, KT, HPC * VW], F16),
        ("bq", [P, 2], F32), ("bk", [P, 2], F32),
        ("bv", [1, HPC * VW], F16), ("wp", [P, 2, D], F16),
        ("tri", [P, P], F16), ("ones1", [1, P], F16),
    ):
        ap[name] = nc.dram_tensor(name, shape, dt, kind="ExternalInput").ap()
    ap["out"] = nc.dram_tensor("out", [S, D], F16, kind="ExternalOutput").ap()

    with tile.TileContext(nc) as tc:
        emit_kernel(nc, tc, ap)
    nc.compile()
    return nc


def make_core_inputs(hidden_states, w_attn, b_attn, w_proj):
    """Host-side sharding: per-core input dicts (core = batch*4 + head_group)."""
    f16, f32 = np.float16, np.float32
    x = np.asarray(hidden_states, f32)
    w_attn = np.asarray(w_attn, f32)
    b_attn = np.asarray(b_attn, f32)
    w_proj = np.asarray(w_proj, f32)

    tri = (np.arange(P)[:, None] <= np.arange(P)[None, :]).astype(f16)
    ones_row = np.ones((1, P), f16)

    def kmaj(w):
        # [D=(KT P), C] -> [P, KT, C] contiguous (the SBUF tile layout)
        return np.ascontiguousarray(
            w.reshape(KT, P, -1).transpose(1, 0, 2)).astype(f16)

    # x.T [(KT P), (NSQ SQ)] -> [P, NSQ, KT, SQ] contiguous seq-quarters
    xTs = [np.ascontiguousarray(
        x[b].T.reshape(KT, P, NSQ, SQ).transpose(1, 2, 0, 3)).astype(f16)
        for b in range(B)]

    in_maps = []
    for core in range(NCORES):
        b, g = core // HPC, core % HPC
        wq = kmaj(w_attn[:, g * CH:(g + 1) * CH])
        wk = kmaj(w_attn[:, D + g * CH:D + (g + 1) * CH])
        wv = np.zeros((D, HPC * VW), np.float32)
        bv = np.zeros((1, HPC * VW), f16)
        for h in range(HPC):
            src = 2 * D + (g * HPC + h) * HD
            wv[:, h * VW:h * VW + HD] = w_attn[:, src:src + HD]
            bv[0, h * VW:h * VW + HD] = b_attn[src:src + HD]
            bv[0, h * VW + HD] = 1.0
        wv = kmaj(wv)
        bq = np.ascontiguousarray(
            b_attn[g * CH:(g + 1) * CH].reshape(2, P).T)
        bk = np.ascontiguousarray(
            b_attn[D + g * CH:D + (g + 1) * CH].reshape(2, P).T)
        wp = np.ascontiguousarray(
            w_proj[g * CH:(g + 1) * CH, :].reshape(2, P, D)
            .transpose(1, 0, 2)).astype(f16)
        in_maps.append({
            "xT": xTs[b], "wq": wq, "wk": wk, "wv": wv,
            "bq": bq, "bk": bk, "bv": bv, "wp": wp,
            "tri": tri, "ones1": ones_row,
        })
    return in_maps


_PROGRAM = None


def kernel(hidden_states, w_attn, b_attn, w_proj, b_proj):
    global _PROGRAM
    if _PROGRAM is None:
        _PROGRAM = build_program()
    in_maps = make_core_inputs(hidden_states, w_attn, b_attn, w_proj)
    res = run_bass_kernel_spmd(_PROGRAM, in_maps, core_ids=list(range(NCORES)))
    out = np.zeros((B, S, D), np.float32)
    for core in range(NCORES):
        out[core // HPC] += res.results[core]["out"].astype(np.float32)
    out += np.asarray(b_proj, np.float32)
    return out


# revision 23
# speedup vs baseline: 1.2398x; 1.0042x over previous
"""Fused causal multi-head attention block on 8 Trainium2 NeuronCores.

Problem (GPT-2 style attention, B=2, S=2048, D=1024, H=16, hd=64):
    qkv = x @ w_attn + b_attn ; split q,k,v ; per-head causal softmax(q k^T / 8) v
    out = attn_out @ w_proj + b_proj

Sharding: data parallel on batch (2) x tensor parallel on heads (4 groups of 4
heads). Core c -> batch c//4, head group c%4. Each core computes a partial
[S, D] output (its heads' slice of w_proj rows); host sums the 4 partials per
batch and adds b_proj.

Per-core kernel layout tricks:
- scores are computed TRANSPOSED (scoresT[key, query]) so the softmax
  denominator falls out of the attn@v matmul by appending a ones-column to v:
  [v | 1]^T @ exp(scoresT) yields the unnormalized output and the per-query
  denominator in one PSUM accumulation.
- matmul inputs are fp16 (full PE rate + fast weight loads); all accumulation
  is fp32 in PSUM. exp(s/8) is in [0, ~13], well inside fp16 range.
- back-to-back matmuls accumulating into the SAME PSUM region serialize on
  the array drain (~175ns each). All contraction chains (qkv, proj) are
  emitted as interleaved PAIRS of chains targeting two PSUM regions, so one
  chain's fill overlaps the other's drain.
- causal masking: fully-masked blocks are skipped via restricted matmul
  widths; diagonal blocks get their masked triangle zeroed on the (otherwise
  idle) GpSimd engine, keeping both the PE and the vector engine out of the
  score->attnv chain.
- x is DMA'd in 4 seq-quarters so QKV chunk c only waits for quarter c;
  weight DMAs are ordered ahead of the x quarter they gate. Small consts go
  out on the gpsimd SWDGE ring in parallel with the sync ring.
- emission is chunk-pipelined: QKV chunk c+1, proj of completed chunks, and
  the previous pair's normalization are spread as PE filler between the
  exp-paced attnv groups, so the PE has dense matmul work while ScalarE runs
  exp. Two fillers are popped before the first attnv group to cover the
  ScalarE backlog at pair transitions.
- the two per-pair score matmuls (K=64 at partition bases 0/64) row-tile into
  the PE array concurrently; the two norm broadcast matmuls (M=64 at output
  bases 0/64) col-tile concurrently.
"""

import sys

sys.path.insert(0, "/opt/trn_rl_repo")

import numpy as np

import concourse.bass as bass
import concourse.mybir as mybir
import concourse.tile as tile
from concourse import bacc
from concourse.bass_utils import run_bass_kernel_spmd

F32 = mybir.dt.float32
F16 = mybir.dt.float16
AFT = mybir.ActivationFunctionType

B, S, D, H, HD = 2, 2048, 1024, 16, 64
NCORES = 8
HPC = 4            # heads per core
CH = HPC * HD      # 256 channels per core
VW = HD + 1        # v width incl. ones column
P = 128
KT = D // P        # 8 contraction tiles over D
SQ = 512           # query/N chunk
NSQ = S // SQ      # 4
NST = S // P       # 16 seq tiles
SCALE = 1.0 / np.sqrt(HD)


def emit_kernel(nc, tc, ap):
    """Emit the per-core program. `ap` is a dict of DRAM APs."""
    with (
        tc.tile_pool(name="const", bufs=1) as cp,
        tc.tile_pool(name="xw", bufs=1) as xw,
        tc.tile_pool(name="act", bufs=1) as acts,
        tc.tile_pool(name="ex", bufs=16) as exp_pool,
        tc.tile_pool(name="dh", bufs=4) as dh_pool,
        tc.tile_pool(name="rc", bufs=2) as rc_pool,
        tc.tile_pool(name="osb", bufs=3) as osb,
        tc.tile_pool(name="psA", bufs=2, space="PSUM") as psA,
        tc.tile_pool(name="psB", bufs=2, space="PSUM") as psB,
        tc.tile_pool(name="psC", bufs=2, space="PSUM") as psC,
    ):
        # ---- weight/x DMAs on the sync (HWDGE) ring, in consumption order.
        # All sources are host-prepared in the exact SBUF layout (contiguous
        # per-partition blocks) so the DMAs run at line rate. x comes in 4
        # seq-quarters so QKV chunk c is gated only on quarter c.
        # everything streams on the single sync HWDGE ring in consumption
        # order: the SDMA engines round-robin across rings with no priority,
        # so a second ring would steal HBM bandwidth from the gating
        # transfers at the front of this one
        wq = xw.tile([P, KT, CH], F16, name="wq", tag="wq")
        nc.sync.dma_start(wq, ap["wq"])
        wk = xw.tile([P, KT, CH], F16, name="wk", tag="wk")
        nc.sync.dma_start(wk, ap["wk"])
        bq = cp.tile([P, 2], F32, name="bq", tag="bq")
        nc.sync.dma_start(bq, ap["bq"])
        bk = cp.tile([P, 2], F32, name="bk", tag="bk")
        nc.sync.dma_start(bk, ap["bk"])
        xts = xw.tile([P, KT, S], F16, name="xts", tag="xts")
        for c in range(NSQ):
            nc.sync.dma_start(xts[:, :, c * SQ:(c + 1) * SQ], ap["xT"][:, c])
            if c == 0:
                wv = xw.tile([P, KT, HPC * VW], F16, name="wv", tag="wv")
                nc.sync.dma_start(wv, ap["wv"])
        wp = xw.tile([P, 2, D], F16, name="wp", tag="wp")
        nc.sync.dma_start(wp, ap["wp"])

        # warmup scratch zeroed on gpsimd BEFORE its const DMAs queue up
        # (and not on the vector engine, whose preamble lands ~5us in)
        wsrc = cp.tile([P, SQ], F16, name="wsrc", tag="wsrc")
        nc.gpsimd.memset(wsrc, 0.0)

        # small consts on the gpsimd SWDGE ring, in parallel with the above
        ones1 = cp.tile([1, P], F16, name="ones1", tag="ones1")
        nc.gpsimd.dma_start(ones1, ap["ones1"])
        bv = cp.tile([1, HPC * VW], F16, name="bv", tag="bv")
        nc.gpsimd.dma_start(bv, ap["bv"])
        tri = cp.tile([P, P], F16, name="tri", tag="tri")
        nc.gpsimd.dma_start(tri, ap["tri"])

        # ---- PE warmup: dense dummy matmuls while input DMAs stream in.
        # The PE clock-gate (HAM) unthrottles 1.2->2.4 GHz only after ~3.4us
        # of sustained matmul activity; burn that in on scratch data.
        # 26 matmuls cover the ~7us sync-queue preamble + the first weight/x
        # transfers at the cold (1.2 GHz) rate, so the PE is warm and fed
        # when the first real matmul's inputs land.
        wps = psB.tile([P, SQ], F32, name="wps", tag="acc")
        for i in range(26):
            nc.tensor.matmul(
                wps, wsrc[:, 0:P], wsrc, start=(i == 0), stop=(i == 25),
            )

        xts_k = [xts[:, k, :] for k in range(KT)]
        wq_t = [wq[:, k, :] for k in range(KT)]
        wk_t = [wk[:, k, :] for k in range(KT)]
        wv_t = [wv[:, k, :] for k in range(KT)]
        wp_t = [wp[:, k, :] for k in range(2)]

        # ---- activations living across phases ----
        qT = acts.tile([P, 2, S], F16, name="qT", tag="qT")
        kTt = acts.tile([P, 2, S], F16, name="kT", tag="kT")
        vv = acts.tile([P, NST, HPC * VW], F16, name="vv", tag="vv")
        outT = [acts.tile([P, S], F16, name=f"oT{i}", tag=f"oT{i}") for i in range(2)]

        def qk_pair(c, dst, wt, bias, pool):
            """Both i-halves of a q or k projection chunk, as interleaved
            chains into two PSUM regions (psC: one 2-bank tile; psA: two
            1-bank tiles) so consecutive accumulates hit different banks."""
            if pool is psC:
                ps2 = psC.tile([P, 2, SQ], F32, name="sc2", tag="sc")
                pss = [ps2[:, 0, :], ps2[:, 1, :]]
            else:
                pss = [psA.tile([P, SQ], F32, name="ps", tag="ps")
                       for _ in range(2)]
            for k in range(KT):
                for i in range(2):
                    nc.tensor.matmul(
                        pss[i],
                        wt[k][:, i * P:(i + 1) * P],
                        xts_k[k][:, c * SQ:(c + 1) * SQ],
                        start=(k == 0),
                        stop=(k == KT - 1),
                    )
            with nc.allow_low_precision(reason="fp16 matmul inputs"):
                if pool is psC:
                    nc.vector.tensor_add(
                        dst[:, :, c * SQ:(c + 1) * SQ], ps2,
                        bias[:, :, None].broadcast_to([P, 2, SQ]),
                    )
                else:
                    for i in range(2):
                        nc.vector.tensor_scalar_add(
                            dst[:, i, c * SQ:(c + 1) * SQ], pss[i],
                            bias[:, i:i + 1],
                        )

        def v_pair(st0, pool):
            """Two v seq-tiles as interleaved chains (natural layout +
            interleaved ones cols; the trailing ones matmul adds v-bias and
            the denominator ones column)."""
            if pool is psC:
                ps2 = psC.tile([P, 2, SQ], F32, name="sc2", tag="sc")
                pss = [ps2[:, 0, 0:HPC * VW], ps2[:, 1, 0:HPC * VW]]
            else:
                pss = [psA.tile([P, SQ], F32, name="psv", tag="ps")[:, 0:HPC * VW]
                       for _ in range(2)]
            for k in range(KT):
                for ci in range(2):
                    nc.tensor.matmul(
                        pss[ci],
                        xts_k[k][:, (st0 + ci) * P:(st0 + ci + 1) * P],
                        wv_t[k],
                        start=(k == 0),
                        stop=False,
                    )
            for ci in range(2):
                nc.tensor.matmul(pss[ci], ones1, bv, start=False, stop=True)
            with nc.allow_low_precision(reason="fp16 matmul inputs"):
                for ci in range(2):
                    nc.vector.tensor_copy(vv[:, st0 + ci, :], pss[ci])

        def qkv_groups(c, pool):
            yield lambda: qk_pair(c, qT, wq_t, bq, pool)
            yield lambda: qk_pair(c, kTt, wk_t, bk, pool)
            yield lambda: v_pair(4 * c, pool)
            yield lambda: v_pair(4 * c + 2, pool)

        def attention_pair(i, c, fillers=(), tail=False):
            """Heads 2i (kT/qT partition rows 0:64) and 2i+1 (rows 64:128).

            Both heads' scores for a key tile land in one 2-bank PSUM tile so
            a single exp instruction covers them (halves ScalarE instruction
            count). All scores are emitted before all attnv matmuls: the PE
            stream is in-order, so ScalarE's exps pipeline behind the score
            stream. The PE stalls on exp pacing both in the scores loop (sc
            PSUM rotation) and the attnv loop, so fillers pop on a schedule
            spread across BOTH loops. Diagonal blocks get their masked
            triangle zeroed by a GpSimd multiply with the tri mask."""
            nkt = 4 * (c + 1)
            accs = [psB.tile([VW, SQ], F32, name="acc", tag="acc")
                    for _ in range(2)]
            fillers = list(fillers)
            nf = len(fillers)
            steps = 2 * nkt

            def pop_due(step):
                while fillers and len(fillers) > nf * (steps - 1 - step) // steps:
                    fillers.pop(0)()

            exs = []
            for kt in range(nkt):
                colo = max(0, kt * P - c * SQ)
                diag = colo > 0 or kt * P == c * SQ
                sc2 = psC.tile([P, 2, SQ], F32, name="sc2", tag="sc")
                for j in range(2):
                    ro = j * 64
                    nc.tensor.matmul(
                        sc2[:, j, colo:SQ],
                        kTt[ro:ro + 64, i, kt * P:(kt + 1) * P],
                        qT[ro:ro + 64, i, c * SQ + colo:(c + 1) * SQ],
                        start=True,
                        stop=True,
                    )
                ex2 = exp_pool.tile([P, 2, SQ], F16, name="ex2", tag="ex")
                nc.scalar.activation(
                    ex2[:, :, colo:SQ], sc2[:, :, colo:SQ], AFT.Exp, scale=SCALE,
                )
                if diag:
                    # zero the masked triangle of the diagonal block on the
                    # (otherwise idle) GpSimd engine
                    nc.gpsimd.tensor_mul(
                        ex2[:, :, colo:colo + P],
                        ex2[:, :, colo:colo + P],
                        tri[:, None, :].broadcast_to([P, 2, P]),
                    )
                exs.append((ex2, kt, colo))
                # the scores loop is itself exp-paced (the sc PSUM slots
                # recycle only as ScalarE drains), so fillers pop here too
                if kt >= 2:
                    pop_due(kt - 2)
            for ex2, kt, colo in exs:
                for j in range(2):
                    h = 2 * i + j
                    nc.tensor.matmul(
                        accs[j][:, colo:SQ],
                        vv[:, kt, h * VW:(h + 1) * VW],
                        ex2[:, j, colo:SQ],
                        start=(kt == 0),
                        stop=(kt == nkt - 1),
                    )
                # dense PE filler between exp-paced attnv groups
                pop_due(nkt - 2 + kt)
            # the tiny denominator copies go first so the norm chain
            # (db matmuls -> reciprocal) starts as early as possible
            dns = []
            for j in range(2):
                dn = dh_pool.tile([1, SQ], F16, name="dn", tag="dn")
                with nc.allow_low_precision(reason="fp16 matmul inputs"):
                    nc.vector.tensor_copy(dn, accs[j][64:65, :])
                dns.append(dn)
            for j in range(2):
                # at the tail ScalarE is idle: move the big outT copies there
                # so the DVE queue reaches the reciprocal/normalize sooner
                with nc.allow_low_precision(reason="fp16 matmul inputs"):
                    dst = outT[i][j * 64:j * 64 + 64, c * SQ:(c + 1) * SQ]
                    if tail:
                        nc.scalar.copy(dst, accs[j][0:64, :])
                    else:
                        nc.vector.tensor_copy(dst, accs[j][0:64, :])
            return dns

        def norm_pair(c, i, dns, pool=None):
            # outT *= 1/denominator: broadcast denoms via K=1 matmuls (the
            # two M=64 matmuls col-tile concurrently via base_partition),
            # one 128-lane fast reciprocal, one fp16 multiply
            db = (pool or psA).tile([P, SQ], F32, name="db",
                                    tag="acc" if pool is psB else "ps")
            nc.tensor.matmul(
                db[0:64, :], ones1[:, 0:64], dns[0],
                start=True, stop=True,
            )
            nc.tensor.matmul(
                db[64:P, :], ones1[:, 0:64], dns[1],
                start=True, stop=True,
            )
            rc32 = rc_pool.tile([P, SQ], F32, name="rc32", tag="rc32")
            nc.vector.reciprocal_approx_fast(rc32, db)
            with nc.allow_low_precision(reason="fp16 matmul inputs"):
                nc.vector.tensor_mul(
                    outT[i][:, c * SQ:(c + 1) * SQ],
                    outT[i][:, c * SQ:(c + 1) * SQ],
                    rc32,
                )

        def proj_mtile(m, tail=False):
            # one m-tile of the projection: [128 seq, 1024 outdims]; the two
            # nch contraction chains interleave across two PSUM tiles, and
            # both halves land in one ob tile -> a single 256KB out DMA
            ob = osb.tile([P, 2 * SQ], F16, name="ob", tag="ob")
            pss = [psA.tile([P, SQ], F32, name="ps", tag="ps")
                   for _ in range(2)]
            for kk in range(2):
                for nch in range(2):
                    nc.tensor.matmul(
                        pss[nch],
                        outT[kk][:, m * P:(m + 1) * P],
                        wp_t[kk][:, nch * SQ:(nch + 1) * SQ],
                        start=(kk == 0),
                        stop=(kk == 1),
                    )
            for nch in range(2):
                with nc.allow_low_precision(reason="partial sums; host sums fp32"):
                    # at the tail ScalarE is idle: split the two copies
                    # across ScalarE and DVE so they run concurrently
                    if tail and nch == 0:
                        nc.scalar.copy(ob[:, 0:SQ], pss[0])
                    else:
                        nc.vector.tensor_copy(
                            ob[:, nch * SQ:(nch + 1) * SQ], pss[nch])
                if tail:
                    # half-tile DMAs off alternating rings so the last
                    # transfer is small and starts as early as possible
                    eng = nc.gpsimd if nch == 0 else nc.sync
                    eng.dma_start(
                        ap["out"][m * P:(m + 1) * P, nch * SQ:(nch + 1) * SQ],
                        ob[:, nch * SQ:(nch + 1) * SQ])
            if not tail:
                nc.sync.dma_start(ap["out"][m * P:(m + 1) * P, :], ob)

        # ---- chunk-pipelined main body ----
        # chunk 0 QKV upfront through the score-PSUM pool (scores aren't
        # running yet), with warmup matmuls between the early groups to keep
        # the PE clock-gate busy while the x quarters stream in
        for gi, g in enumerate(qkv_groups(0, psC)):
            g()
            if gi < 2:
                for i in range(2):
                    nc.tensor.matmul(
                        wps, wsrc[:, 0:P], wsrc,
                        start=(i == 0), stop=(i == 1),
                    )
        # attention(c) runs against qkv chunks emitted one chunk ahead.
        # Filler slots between the exp-paced attnv groups carry, in order:
        # the previous pair's norm, the next chunk's qkv groups, and proj
        # m-tiles of chunks whose norms are complete. Chunk t's proj
        # (m=4t..4t+3) becomes eligible after norm(t,1).
        # per-pair filler budget tracks the pair's exp-pacing deficit
        # (~283ns x nkt + transition); proj tiles m0-m5 are deferred to the
        # last pair, which otherwise has no eligible work left and starves
        filler_plan = {
            (0, 0): [],
            (0, 1): ["n00"],
            (1, 0): ["n01"],
            (1, 1): ["n10"],
            (2, 0): ["n11"],
            (2, 1): ["n20"],
            (3, 0): ["n21", "m6", "m7", "m8", "m9", "m10", "m11"],
            (3, 1): ["n30", "m0", "m1", "m2", "m3", "m4", "m5"],
        }
        norms = {}          # (c, i) -> dns, filled as pairs complete

        def make_filler(tok):
            if tok.startswith("n"):
                c, i = int(tok[1]), int(tok[2])
                return lambda: norm_pair(c, i, norms[(c, i)])
            m = int(tok[1:])
            return lambda: proj_mtile(m)

        for c in range(NSQ):
            nxt = list(qkv_groups(c + 1, psA)) if c + 1 < NSQ else []
            for i in range(2):
                fillers = [make_filler(t) for t in filler_plan[(c, i)]]
                # next chunk's qkv interleaves after the norm filler
                fillers[1:1] = nxt[2 * i:2 * i + 2]
                norms[(c, i)] = attention_pair(
                    i, c, fillers, tail=(c == NSQ - 1 and i == 1))
        # tail: the last norm's broadcast goes through psB (psA would make
        # the first proj tile wait on the reciprocal), then chunk 3's proj
        norm_pair(NSQ - 1, 1, norms[(NSQ - 1, 1)], pool=psB)
        for m in range(12, 16):
            proj_mtile(m, tail=True)


def build_program():
    nc = bacc.Bacc("TRN2", target_bir_lowering=False, debug=False,
                   num_devices=NCORES)
    ap = {}
    for name, shape, dt in (
        ("xT", [P, NSQ, KT, SQ], F16), ("wq", [P, KT, CH], F16),
        ("wk", [P, KT, CH], F16), ("wv", [P, KT, HPC * VW], F16),
        ("bq", [P, 2], F32), ("bk", [P, 2], F32),
        ("bv", [1, HPC * VW], F16), ("wp", [P, 2, D], F16),
        ("tri", [P, P], F16), ("ones1", [1, P], F16),
    ):
        ap[name] = nc.dram_tensor(name, shape, dt, kind="ExternalInput").ap()
    ap["out"] = nc.dram_tensor("out", [S, D], F16, kind="ExternalOutput").ap()

    with tile.TileContext(nc) as tc:
        emit_kernel(nc, tc, ap)
    nc.compile()
    return nc


def make_core_inputs(hidden_states, w_attn, b_attn, w_proj):
    """Host-side sharding: per-core input dicts (core = batch*4 + head_group)."""
    f16, f32 = np.float16, np.float32
    x = np.asarray(hidden_states, f32)
    w_attn = np.asarray(w_attn, f32)
    b_attn = np.asarray(b_attn, f32)
    w_proj = np.asarray(w_proj, f32)

    tri = (np.arange(P)[:, None] <= np.arange(P)[None, :]).astype(f16)
    ones_row = np.ones((1, P), f16)

    def kmaj(w):
        # [D=(KT P), C] -> [P, KT, C] contiguous (the SBUF tile layout)
        return np.ascontiguousarray(
            w.reshape(KT, P, -1).transpose(1, 0, 2)).astype(f16)

    # x.T [(KT P), (NSQ SQ)] -> [P, NSQ, KT, SQ] contiguous seq-quarters
    xTs = [np.ascontiguousarray(
        x[b].T.reshape(KT, P, NSQ, SQ).transpose(1, 2, 0, 3)).astype(f16)
        for b in range(B)]

    in_maps = []
    for core in range(NCORES):
        b, g = core // HPC, core % HPC
        wq = kmaj(w_attn[:, g * CH:(g + 1) * CH])
        wk = kmaj(w_attn[:, D + g * CH:D + (g + 1) * CH])
        wv = np.zeros((D, HPC * VW), np.float32)
        bv = np.zeros((1, HPC * VW), f16)
        for h in range(HPC):
            src = 2 * D + (g * HPC + h) * HD
            wv[:, h * VW:h * VW + HD] = w_attn[:, src:src + HD]
            bv[0, h * VW:h * VW + HD] = b_attn[src:src + HD]
            bv[0, h * VW + HD] = 1.0
        wv = kmaj(wv)
        bq = np.ascontiguousarray(
            b_attn[g * CH:(g + 1) * CH].reshape(2, P).T)
        bk = np.ascontiguousarray(
            b_attn[D + g * CH:D + (g + 1) * CH].reshape(2, P).T)
        wp = np.ascontiguousarray(
            w_proj[g * CH:(g + 1) * CH, :].reshape(2, P, D)
            .transpose(1, 0, 2)).astype(f16)
        in_maps.append({
            "xT": xTs[b], "wq": wq, "wk": wk, "wv": wv,
            "bq": bq, "bk": bk, "bv": bv, "wp": wp,
            "tri": tri, "ones1": ones_row,
        })
    return in_maps


_PROGRAM = None


def kernel(hidden_states, w_attn, b_attn, w_proj, b_proj):
    global _PROGRAM
    if _PROGRAM is None:
        _PROGRAM = build_program()
    in_maps = make_core_inputs(hidden_states, w_attn, b_attn, w_proj)
    res = run_bass_kernel_spmd(_PROGRAM, in_maps, core_ids=list(range(NCORES)))
    out = np.zeros((B, S, D), np.float32)
    for core in range(NCORES):
        out[core // HPC] += res.results[core]["out"].astype(np.float32)
    out += np.asarray(b_proj, np.float32)
    return out


# revision 25
# speedup vs baseline: 1.2538x; 1.0113x over previous
"""Fused causal multi-head attention block on 8 Trainium2 NeuronCores.

Problem (GPT-2 style attention, B=2, S=2048, D=1024, H=16, hd=64):
    qkv = x @ w_attn + b_attn ; split q,k,v ; per-head causal softmax(q k^T / 8) v
    out = attn_out @ w_proj + b_proj

Sharding: data parallel on batch (2) x tensor parallel on heads (4 groups of 4
heads). Core c -> batch c//4, head group c%4. Each core computes a partial
[S, D] output (its heads' slice of w_proj rows); host sums the 4 partials per
batch and adds b_proj.

Per-core kernel layout tricks:
- scores are computed TRANSPOSED (scoresT[key, query]) so the softmax
  denominator falls out of the attn@v matmul by appending a ones-column to v:
  [v | 1]^T @ exp(scoresT) yields the unnormalized output and the per-query
  denominator in one PSUM accumulation.
- matmul inputs are fp16 (full PE rate + fast weight loads); all accumulation
  is fp32 in PSUM. exp(s/8) is in [0, ~13], well inside fp16 range.
- back-to-back matmuls accumulating into the SAME PSUM region serialize on
  the array drain (~175ns each). All contraction chains (qkv, proj) are
  emitted as interleaved PAIRS of chains targeting two PSUM regions, so one
  chain's fill overlaps the other's drain.
- causal masking: fully-masked blocks are skipped via restricted matmul
  widths; diagonal blocks get their masked triangle zeroed on the (otherwise
  idle) GpSimd engine, keeping both the PE and the vector engine out of the
  score->attnv chain.
- x is DMA'd in 4 seq-quarters so QKV chunk c only waits for quarter c;
  weight DMAs are ordered ahead of the x quarter they gate. Small consts go
  out on the gpsimd SWDGE ring in parallel with the sync ring.
- emission is chunk-pipelined: QKV chunk c+1, proj of completed chunks, and
  the previous pair's normalization are spread as PE filler between the
  exp-paced attnv groups, so the PE has dense matmul work while ScalarE runs
  exp. Two fillers are popped before the first attnv group to cover the
  ScalarE backlog at pair transitions.
- the two per-pair score matmuls (K=64 at partition bases 0/64) row-tile into
  the PE array concurrently; the two norm broadcast matmuls (M=64 at output
  bases 0/64) col-tile concurrently.
"""

import sys

sys.path.insert(0, "/opt/trn_rl_repo")

import numpy as np

import concourse.bass as bass
import concourse.mybir as mybir
import concourse.tile as tile
from concourse import bacc
from concourse.bass_utils import run_bass_kernel_spmd

F32 = mybir.dt.float32
F16 = mybir.dt.float16
AFT = mybir.ActivationFunctionType

B, S, D, H, HD = 2, 2048, 1024, 16, 64
NCORES = 8
HPC = 4            # heads per core
CH = HPC * HD      # 256 channels per core
VW = HD + 1        # v width incl. ones column
P = 128
KT = D // P        # 8 contraction tiles over D
SQ = 512           # query/N chunk
NSQ = S // SQ      # 4
NST = S // P       # 16 seq tiles
SCALE = 1.0 / np.sqrt(HD)


def emit_kernel(nc, tc, ap):
    """Emit the per-core program. `ap` is a dict of DRAM APs."""
    with (
        tc.tile_pool(name="const", bufs=1) as cp,
        tc.tile_pool(name="xw", bufs=1) as xw,
        tc.tile_pool(name="act", bufs=1) as acts,
        tc.tile_pool(name="ex", bufs=16) as exp_pool,
        tc.tile_pool(name="dh", bufs=4) as dh_pool,
        tc.tile_pool(name="rc", bufs=2) as rc_pool,
        tc.tile_pool(name="osb", bufs=3) as osb,
        tc.tile_pool(name="psA", bufs=2, space="PSUM") as psA,
        tc.tile_pool(name="psB", bufs=2, space="PSUM") as psB,
        tc.tile_pool(name="psC", bufs=2, space="PSUM") as psC,
    ):
        # ---- weight/x DMAs on the sync (HWDGE) ring, in consumption order.
        # All sources are host-prepared in the exact SBUF layout (contiguous
        # per-partition blocks) so the DMAs run at line rate. x comes in 4
        # seq-quarters so QKV chunk c is gated only on quarter c.
        # everything streams on the single sync HWDGE ring in consumption
        # order: the SDMA engines round-robin across rings with no priority,
        # so a second ring would steal HBM bandwidth from the gating
        # transfers at the front of this one
        # wq + x quarter 0 gate the first real matmul; wk/bq/bk are not
        # needed until a few us later, so they follow
        wq = xw.tile([P, KT, CH], F16, name="wq", tag="wq")
        nc.sync.dma_start(wq, ap["wq"])
        xts = xw.tile([P, KT, S], F16, name="xts", tag="xts")
        nc.sync.dma_start(xts[:, :, 0:SQ], ap["xT"][:, 0])
        wk = xw.tile([P, KT, CH], F16, name="wk", tag="wk")
        nc.sync.dma_start(wk, ap["wk"])
        bq = cp.tile([P, 2], F32, name="bq", tag="bq")
        nc.sync.dma_start(bq, ap["bq"])
        bk = cp.tile([P, 2], F32, name="bk", tag="bk")
        nc.sync.dma_start(bk, ap["bk"])
        wv = xw.tile([P, KT, HPC * VW], F16, name="wv", tag="wv")
        nc.sync.dma_start(wv, ap["wv"])
        for c in range(1, NSQ):
            nc.sync.dma_start(xts[:, :, c * SQ:(c + 1) * SQ], ap["xT"][:, c])
        wp = xw.tile([P, 2, D], F16, name="wp", tag="wp")
        nc.sync.dma_start(wp, ap["wp"])

        # warmup scratch zeroed on gpsimd BEFORE its const DMAs queue up
        # (and not on the vector engine, whose preamble lands ~5us in)
        wsrc = cp.tile([P, SQ], F16, name="wsrc", tag="wsrc")
        nc.gpsimd.memset(wsrc, 0.0)

        # small consts on the gpsimd SWDGE ring, in parallel with the above
        ones1 = cp.tile([1, P], F16, name="ones1", tag="ones1")
        nc.gpsimd.dma_start(ones1, ap["ones1"])
        bv = cp.tile([1, HPC * VW], F16, name="bv", tag="bv")
        nc.gpsimd.dma_start(bv, ap["bv"])
        tri = cp.tile([P, P], F16, name="tri", tag="tri")
        nc.gpsimd.dma_start(tri, ap["tri"])

        # ---- PE warmup: dense dummy matmuls while input DMAs stream in.
        # The PE clock-gate (HAM) unthrottles 1.2->2.4 GHz only after ~3.4us
        # of sustained matmul activity; burn that in on scratch data.
        # 26 matmuls cover the ~7us sync-queue preamble + the first weight/x
        # transfers at the cold (1.2 GHz) rate, so the PE is warm and fed
        # when the first real matmul's inputs land.
        wps = psB.tile([P, SQ], F32, name="wps", tag="acc")
        for i in range(23):
            nc.tensor.matmul(
                wps, wsrc[:, 0:P], wsrc, start=(i == 0), stop=(i == 22),
            )

        xts_k = [xts[:, k, :] for k in range(KT)]
        wq_t = [wq[:, k, :] for k in range(KT)]
        wk_t = [wk[:, k, :] for k in range(KT)]
        wv_t = [wv[:, k, :] for k in range(KT)]
        wp_t = [wp[:, k, :] for k in range(2)]

        # ---- activations living across phases ----
        qT = acts.tile([P, 2, S], F16, name="qT", tag="qT")
        kTt = acts.tile([P, 2, S], F16, name="kT", tag="kT")
        vv = acts.tile([P, NST, HPC * VW], F16, name="vv", tag="vv")
        outT = [acts.tile([P, S], F16, name=f"oT{i}", tag=f"oT{i}") for i in range(2)]

        def qk_pair(c, dst, wt, bias, pool):
            """Both i-halves of a q or k projection chunk, as interleaved
            chains into two PSUM regions (psC: one 2-bank tile; psA: two
            1-bank tiles) so consecutive accumulates hit different banks."""
            if pool is psC:
                ps2 = psC.tile([P, 2, SQ], F32, name="sc2", tag="sc")
                pss = [ps2[:, 0, :], ps2[:, 1, :]]
            else:
                pss = [psA.tile([P, SQ], F32, name="ps", tag="ps")
                       for _ in range(2)]
            for k in range(KT):
                for i in range(2):
                    nc.tensor.matmul(
                        pss[i],
                        wt[k][:, i * P:(i + 1) * P],
                        xts_k[k][:, c * SQ:(c + 1) * SQ],
                        start=(k == 0),
                        stop=(k == KT - 1),
                    )
            with nc.allow_low_precision(reason="fp16 matmul inputs"):
                if pool is psC:
                    nc.vector.tensor_add(
                        dst[:, :, c * SQ:(c + 1) * SQ], ps2,
                        bias[:, :, None].broadcast_to([P, 2, SQ]),
                    )
                else:
                    for i in range(2):
                        nc.vector.tensor_scalar_add(
                            dst[:, i, c * SQ:(c + 1) * SQ], pss[i],
                            bias[:, i:i + 1],
                        )

        def v_pair(st0, pool):
            """Two v seq-tiles as interleaved chains (natural layout +
            interleaved ones cols; the trailing ones matmul adds v-bias and
            the denominator ones column)."""
            if pool is psC:
                ps2 = psC.tile([P, 2, SQ], F32, name="sc2", tag="sc")
                pss = [ps2[:, 0, 0:HPC * VW], ps2[:, 1, 0:HPC * VW]]
            else:
                pss = [psA.tile([P, SQ], F32, name="psv", tag="ps")[:, 0:HPC * VW]
                       for _ in range(2)]
            for k in range(KT):
                for ci in range(2):
                    nc.tensor.matmul(
                        pss[ci],
                        xts_k[k][:, (st0 + ci) * P:(st0 + ci + 1) * P],
                        wv_t[k],
                        start=(k == 0),
                        stop=False,
                    )
            for ci in range(2):
                nc.tensor.matmul(pss[ci], ones1, bv, start=False, stop=True)
            with nc.allow_low_precision(reason="fp16 matmul inputs"):
                for ci in range(2):
                    nc.vector.tensor_copy(vv[:, st0 + ci, :], pss[ci])

        def qkv_groups(c, pool):
            yield lambda: qk_pair(c, qT, wq_t, bq, pool)
            yield lambda: qk_pair(c, kTt, wk_t, bk, pool)
            yield lambda: v_pair(4 * c, pool)
            yield lambda: v_pair(4 * c + 2, pool)

        def attention_pair(i, c, fillers=(), tail=False):
            """Heads 2i (kT/qT partition rows 0:64) and 2i+1 (rows 64:128).

            Both heads' scores for a key tile land in one 2-bank PSUM tile so
            a single exp instruction covers them (halves ScalarE instruction
            count). All scores are emitted before all attnv matmuls: the PE
            stream is in-order, so ScalarE's exps pipeline behind the score
            stream. The PE stalls on exp pacing both in the scores loop (sc
            PSUM rotation) and the attnv loop, so fillers pop on a schedule
            spread across BOTH loops. Diagonal blocks get their masked
            triangle zeroed by a GpSimd multiply with the tri mask."""
            nkt = 4 * (c + 1)
            accs = [psB.tile([VW, SQ], F32, name="acc", tag="acc")
                    for _ in range(2)]
            fillers = list(fillers)
            nf = len(fillers)
            steps = 2 * nkt

            def pop_due(step):
                while fillers and len(fillers) > nf * (steps - 1 - step) // steps:
                    fillers.pop(0)()

            exs = []
            for kt in range(nkt):
                colo = max(0, kt * P - c * SQ)
                diag = colo > 0 or kt * P == c * SQ
                sc2 = psC.tile([P, 2, SQ], F32, name="sc2", tag="sc")
                for j in range(2):
                    ro = j * 64
                    nc.tensor.matmul(
                        sc2[:, j, colo:SQ],
                        kTt[ro:ro + 64, i, kt * P:(kt + 1) * P],
                        qT[ro:ro + 64, i, c * SQ + colo:(c + 1) * SQ],
                        start=True,
                        stop=True,
                    )
                ex2 = exp_pool.tile([P, 2, SQ], F16, name="ex2", tag="ex")
                nc.scalar.activation(
                    ex2[:, :, colo:SQ], sc2[:, :, colo:SQ], AFT.Exp, scale=SCALE,
                )
                if diag:
                    # zero the masked triangle of the diagonal block on the
                    # (otherwise idle) GpSimd engine
                    nc.gpsimd.tensor_mul(
                        ex2[:, :, colo:colo + P],
                        ex2[:, :, colo:colo + P],
                        tri[:, None, :].broadcast_to([P, 2, P]),
                    )
                exs.append((ex2, kt, colo))
                # the scores loop is itself exp-paced (the sc PSUM slots
                # recycle only as ScalarE drains), so fillers pop here too
                if kt >= 2:
                    pop_due(kt - 2)
            for ex2, kt, colo in exs:
                for j in range(2):
                    h = 2 * i + j
                    nc.tensor.matmul(
                        accs[j][:, colo:SQ],
                        vv[:, kt, h * VW:(h + 1) * VW],
                        ex2[:, j, colo:SQ],
                        start=(kt == 0),
                        stop=(kt == nkt - 1),
                    )
                # dense PE filler between exp-paced attnv groups
                pop_due(nkt - 2 + kt)
            # the tiny denominator copies go first so the norm chain
            # (db matmuls -> reciprocal) starts as early as possible
            dns = []
            for j in range(2):
                dn = dh_pool.tile([1, SQ], F16, name="dn", tag="dn")
                with nc.allow_low_precision(reason="fp16 matmul inputs"):
                    nc.vector.tensor_copy(dn, accs[j][64:65, :])
                dns.append(dn)
            for j in range(2):
                # at the tail ScalarE is idle: move the big outT copies there
                # so the DVE queue reaches the reciprocal/normalize sooner
                with nc.allow_low_precision(reason="fp16 matmul inputs"):
                    dst = outT[i][j * 64:j * 64 + 64, c * SQ:(c + 1) * SQ]
                    if tail:
                        nc.scalar.copy(dst, accs[j][0:64, :])
                    else:
                        nc.vector.tensor_copy(dst, accs[j][0:64, :])
            return dns

        def norm_pair(c, i, dns, pool=None):
            # outT *= 1/denominator: broadcast denoms via K=1 matmuls (the
            # two M=64 matmuls col-tile concurrently via base_partition),
            # one 128-lane fast reciprocal, one fp16 multiply
            db = (pool or psA).tile([P, SQ], F32, name="db",
                                    tag="acc" if pool is psB else "ps")
            nc.tensor.matmul(
                db[0:64, :], ones1[:, 0:64], dns[0],
                start=True, stop=True,
            )
            nc.tensor.matmul(
                db[64:P, :], ones1[:, 0:64], dns[1],
                start=True, stop=True,
            )
            rc32 = rc_pool.tile([P, SQ], F32, name="rc32", tag="rc32")
            nc.vector.reciprocal_approx_fast(rc32, db)
            with nc.allow_low_precision(reason="fp16 matmul inputs"):
                nc.vector.tensor_mul(
                    outT[i][:, c * SQ:(c + 1) * SQ],
                    outT[i][:, c * SQ:(c + 1) * SQ],
                    rc32,
                )

        def proj_mtile(m, tail=False):
            # one m-tile of the projection: [128 seq, 1024 outdims]; the two
            # nch contraction chains interleave across two PSUM tiles, and
            # both halves land in one ob tile -> a single 256KB out DMA
            ob = osb.tile([P, 2 * SQ], F16, name="ob", tag="ob")
            pss = [psA.tile([P, SQ], F32, name="ps", tag="ps")
                   for _ in range(2)]
            for kk in range(2):
                for nch in range(2):
                    nc.tensor.matmul(
                        pss[nch],
                        outT[kk][:, m * P:(m + 1) * P],
                        wp_t[kk][:, nch * SQ:(nch + 1) * SQ],
                        start=(kk == 0),
                        stop=(kk == 1),
                    )
            for nch in range(2):
                with nc.allow_low_precision(reason="partial sums; host sums fp32"):
                    # at the tail ScalarE is idle: split the two copies
                    # across ScalarE and DVE so they run concurrently
                    if tail and nch == 0:
                        nc.scalar.copy(ob[:, 0:SQ], pss[0])
                    else:
                        nc.vector.tensor_copy(
                            ob[:, nch * SQ:(nch + 1) * SQ], pss[nch])
                if tail:
                    # half-tile DMAs off alternating rings so the last
                    # transfer is small and starts as early as possible
                    eng = nc.gpsimd if nch == 0 else nc.sync
                    eng.dma_start(
                        ap["out"][m * P:(m + 1) * P, nch * SQ:(nch + 1) * SQ],
                        ob[:, nch * SQ:(nch + 1) * SQ])
            if not tail:
                nc.sync.dma_start(ap["out"][m * P:(m + 1) * P, :], ob)

        # ---- chunk-pipelined main body ----
        # chunk 0 QKV upfront through the score-PSUM pool (scores aren't
        # running yet), with warmup matmuls between the early groups to keep
        # the PE clock-gate busy while the x quarters stream in
        for gi, g in enumerate(qkv_groups(0, psC)):
            g()
            if gi < 2:
                for i in range(2):
                    nc.tensor.matmul(
                        wps, wsrc[:, 0:P], wsrc,
                        start=(i == 0), stop=(i == 1),
                    )
        # attention(c) runs against qkv chunks emitted one chunk ahead.
        # Filler slots between the exp-paced attnv groups carry, in order:
        # the previous pair's norm, the next chunk's qkv groups, and proj
        # m-tiles of chunks whose norms are complete. Chunk t's proj
        # (m=4t..4t+3) becomes eligible after norm(t,1).
        # per-pair filler budget tracks the pair's exp-pacing deficit
        # (~283ns x nkt + transition); proj tiles m0-m5 are deferred to the
        # last pair, which otherwise has no eligible work left and starves
        filler_plan = {
            (0, 0): [],
            (0, 1): ["n00"],
            (1, 0): ["n01"],
            (1, 1): ["n10"],
            (2, 0): ["n11"],
            (2, 1): ["n20"],
            (3, 0): ["n21", "m6", "m7", "m8", "m9", "m10", "m11"],
            (3, 1): ["n30", "m0", "m1", "m2", "m3", "m4", "m5"],
        }
        norms = {}          # (c, i) -> dns, filled as pairs complete

        def make_filler(tok):
            if tok.startswith("n"):
                c, i = int(tok[1]), int(tok[2])
                return lambda: norm_pair(c, i, norms[(c, i)])
            m = int(tok[1:])
            return lambda: proj_mtile(m)

        for c in range(NSQ):
            nxt = list(qkv_groups(c + 1, psA)) if c + 1 < NSQ else []
            for i in range(2):
                fillers = [make_filler(t) for t in filler_plan[(c, i)]]
                # next chunk's qkv interleaves after the norm filler
                fillers[1:1] = nxt[2 * i:2 * i + 2]
                norms[(c, i)] = attention_pair(
                    i, c, fillers, tail=(c == NSQ - 1 and i == 1))
        # tail: the last norm's broadcast goes through psB (psA would make
        # the first proj tile wait on the reciprocal), then chunk 3's proj
        norm_pair(NSQ - 1, 1, norms[(NSQ - 1, 1)], pool=psB)
        for m in range(12, 16):
            proj_mtile(m, tail=True)


def build_program():
    nc = bacc.Bacc("TRN2", target_bir_lowering=False, debug=False,
                   num_devices=NCORES)
    ap = {}
    for name, shape, dt in (
        ("xT", [P, NSQ, KT, SQ], F16), ("wq", [P, KT, CH], F16),
        ("wk", [P, KT, CH], F16), ("wv", [P, KT, HPC * VW], F16),
        ("bq", [P, 2], F32), ("bk", [P, 2], F32),
        ("bv", [1, HPC * VW], F16), ("wp", [P, 2, D], F16),
        ("tri", [P, P], F16), ("ones1", [1, P], F16),
    ):
        ap[name] = nc.dram_tensor(name, shape, dt, kind="ExternalInput").ap()
    ap["out"] = nc.dram_tensor("out", [S, D], F16, kind="ExternalOutput").ap()

    with tile.TileContext(nc) as tc:
        emit_kernel(nc, tc, ap)
    nc.compile()
    return nc


def make_core_inputs(hidden_states, w_attn, b_attn, w_proj):
    """Host-side sharding: per-core input dicts (core = batch*4 + head_group)."""
    f16, f32 = np.float16, np.float32
    x = np.asarray(hidden_states, f32)
    w_attn = np.asarray(w_attn, f32)
    b_attn = np.asarray(b_attn, f32)
    w_proj = np.asarray(w_proj, f32)

    tri = (np.arange(P)[:, None] <= np.arange(P)[None, :]).astype(f16)
    ones_row = np.ones((1, P), f16)

    def kmaj(w):
        # [D=(KT P), C] -> [P, KT, C] contiguous (the SBUF tile layout)
        return np.ascontiguousarray(
            w.reshape(KT, P, -1).transpose(1, 0, 2)).astype(f16)

    # x.T [(KT P), (NSQ SQ)] -> [P, NSQ, KT, SQ] contiguous seq-quarters
    xTs = [np.ascontiguousarray(
        x[b].T.reshape(KT, P, NSQ, SQ).transpose(1, 2, 0, 3)).astype(f16)
        for b in range(B)]

    in_maps = []
    for core in range(NCORES):
        b, g = core // HPC, core % HPC
        wq = kmaj(w_attn[:, g * CH:(g + 1) * CH])
        wk = kmaj(w_attn[:, D + g * CH:D + (g + 1) * CH])
        wv = np.zeros((D, HPC * VW), np.float32)
        bv = np.zeros((1, HPC * VW), f16)
        for h in range(HPC):
            src = 2 * D + (g * HPC + h) * HD
            wv[:, h * VW:h * VW + HD] = w_attn[:, src:src + HD]
            bv[0, h * VW:h * VW + HD] = b_attn[src:src + HD]
            bv[0, h * VW + HD] = 1.0
        wv = kmaj(wv)
        bq = np.ascontiguousarray(
            b_attn[g * CH:(g + 1) * CH].reshape(2, P).T)
        bk = np.ascontiguousarray(
            b_attn[D + g * CH:D + (g + 1) * CH].reshape(2, P).T)
        wp = np.ascontiguousarray(
            w_proj[g * CH:(g + 1) * CH, :].reshape(2, P, D)
            .transpose(1, 0, 2)).astype(f16)
        in_maps.append({
            "xT": xTs[b], "wq": wq, "wk": wk, "wv": wv,
            "bq": bq, "bk": bk, "bv": bv, "wp": wp,
            "tri": tri, "ones1": ones_row,
        })
    return in_maps


_PROGRAM = None


def kernel(hidden_states, w_attn, b_attn, w_proj, b_proj):
    global _PROGRAM
    if _PROGRAM is None:
        _PROGRAM = build_program()
    in_maps = make_core_inputs(hidden_states, w_attn, b_attn, w_proj)
    res = run_bass_kernel_spmd(_PROGRAM, in_maps, core_ids=list(range(NCORES)))
    out = np.zeros((B, S, D), np.float32)
    for core in range(NCORES):
        out[core // HPC] += res.results[core]["out"].astype(np.float32)
    out += np.asarray(b_proj, np.float32)
    return out


# revision 29
# speedup vs baseline: 1.2651x; 1.0091x over previous
"""Fused causal multi-head attention block on 8 Trainium2 NeuronCores.

Problem (GPT-2 style attention, B=2, S=2048, D=1024, H=16, hd=64):
    qkv = x @ w_attn + b_attn ; split q,k,v ; per-head causal softmax(q k^T / 8) v
    out = attn_out @ w_proj + b_proj

Sharding: data parallel on batch (2) x tensor parallel on heads (4 groups of 4
heads). Core c -> batch c//4, head group c%4. Each core computes a partial
[S, D] output (its heads' slice of w_proj rows); host sums the 4 partials per
batch and adds b_proj.

Per-core kernel layout tricks:
- scores are computed TRANSPOSED (scoresT[key, query]) so the softmax
  denominator falls out of the attn@v matmul by appending a ones-column to v:
  [v | 1]^T @ exp(scoresT) yields the unnormalized output and the per-query
  denominator in one PSUM accumulation.
- matmul inputs are fp16 (full PE rate + fast weight loads); all accumulation
  is fp32 in PSUM. exp(s/8) is in [0, ~13], well inside fp16 range.
- back-to-back matmuls accumulating into the SAME PSUM region serialize on
  the array drain (~175ns each). All contraction chains (qkv, proj) are
  emitted as interleaved PAIRS of chains targeting two PSUM regions, so one
  chain's fill overlaps the other's drain.
- causal masking: fully-masked blocks are skipped via restricted matmul
  widths; diagonal blocks get their masked triangle zeroed on the (otherwise
  idle) GpSimd engine, keeping both the PE and the vector engine out of the
  score->attnv chain.
- x is DMA'd in 4 seq-quarters so QKV chunk c only waits for quarter c; all
  bulk transfers ride one sync-ring queue in exact consumption order (the
  SDMA engines round-robin rings with no priority, so a second ring would
  steal bandwidth from the gating transfers). Small consts go out on the
  gpsimd SWDGE ring. Host pre-arranges every source in its SBUF layout so
  transfers run contiguous at line rate.
- emission is chunk-pipelined: QKV chunk c+1, proj of completed chunks, and
  the previous pair's normalization are spread as PE filler across BOTH the
  score loop (which is exp-paced through the sc-PSUM rotation) and the
  exp-paced attnv groups, sized to each pair's pacing deficit, with proj
  m0-m5 held back for the final pair which otherwise starves.
- the two per-pair score matmuls (K=64 at partition bases 0/64) row-tile
  into the PE array concurrently (verified: second matmul starts 4ns after
  the first); the two norm broadcast matmuls (M=64 at output bases 0/64)
  col-tile concurrently.
"""

import sys

sys.path.insert(0, "/opt/trn_rl_repo")

import numpy as np

import concourse.bass as bass
import concourse.mybir as mybir
import concourse.tile as tile
from concourse import bacc
from concourse.bass_utils import run_bass_kernel_spmd

F32 = mybir.dt.float32
F16 = mybir.dt.float16
AFT = mybir.ActivationFunctionType

B, S, D, H, HD = 2, 2048, 1024, 16, 64
NCORES = 8
HPC = 4            # heads per core
CH = HPC * HD      # 256 channels per core
VW = HD + 1        # v width incl. ones column
P = 128
KT = D // P        # 8 contraction tiles over D
SQ = 512           # query/N chunk
NSQ = S // SQ      # 4
NST = S // P       # 16 seq tiles
SCALE = 1.0 / np.sqrt(HD)


def emit_kernel(nc, tc, ap):
    """Emit the per-core program. `ap` is a dict of DRAM APs."""
    with (
        tc.tile_pool(name="const", bufs=1) as cp,
        tc.tile_pool(name="xw", bufs=1) as xw,
        tc.tile_pool(name="act", bufs=1) as acts,
        tc.tile_pool(name="ex", bufs=16) as exp_pool,
        tc.tile_pool(name="dh", bufs=4) as dh_pool,
        tc.tile_pool(name="rc", bufs=2) as rc_pool,
        tc.tile_pool(name="osb", bufs=3) as osb,
        tc.tile_pool(name="psA", bufs=2, space="PSUM") as psA,
        tc.tile_pool(name="psB", bufs=2, space="PSUM") as psB,
        tc.tile_pool(name="psC", bufs=2, space="PSUM") as psC,
    ):
        # ---- weight/x DMAs on the sync (HWDGE) ring, in consumption order.
        # All sources are host-prepared in the exact SBUF layout (contiguous
        # per-partition blocks) so the DMAs run at line rate. x comes in 4
        # seq-quarters so QKV chunk c is gated only on quarter c.
        # everything streams on the single sync HWDGE ring in consumption
        # order: the SDMA engines round-robin across rings with no priority,
        # so a second ring would steal HBM bandwidth from the gating
        # transfers at the front of this one
        # wq + x quarter 0 gate the first real matmul; both are split in
        # k-halves so the first half of the first contraction chain can
        # start after 0.75MB instead of 1.5MB. wk/bq/bk follow.
        wq = xw.tile([P, KT, CH], F16, name="wq", tag="wq")
        xts = xw.tile([P, KT, S], F16, name="xts", tag="xts")
        hk = KT // 2
        nc.sync.dma_start(wq[:, 0:hk, :], ap["wq"][:, 0:hk, :])
        nc.sync.dma_start(xts[:, 0:hk, 0:SQ], ap["xT"][:, 0, 0:hk, :])
        nc.sync.dma_start(wq[:, hk:KT, :], ap["wq"][:, hk:KT, :])
        nc.sync.dma_start(xts[:, hk:KT, 0:SQ], ap["xT"][:, 0, hk:KT, :])
        wk = xw.tile([P, KT, CH], F16, name="wk", tag="wk")
        nc.sync.dma_start(wk, ap["wk"])
        bq = cp.tile([P, 2], F32, name="bq", tag="bq")
        nc.sync.dma_start(bq, ap["bq"])
        bk = cp.tile([P, 2], F32, name="bk", tag="bk")
        nc.sync.dma_start(bk, ap["bk"])
        wv = xw.tile([P, KT, HPC * VW], F16, name="wv", tag="wv")
        nc.sync.dma_start(wv, ap["wv"])
        for c in range(1, NSQ):
            nc.sync.dma_start(xts[:, :, c * SQ:(c + 1) * SQ], ap["xT"][:, c])
        wp = xw.tile([P, 2, D], F16, name="wp", tag="wp")
        nc.sync.dma_start(wp, ap["wp"])

        # warmup scratch zeroed on gpsimd BEFORE its const DMAs queue up
        # (and not on the vector engine, whose preamble lands ~5us in)
        wsrc = cp.tile([P, SQ], F16, name="wsrc", tag="wsrc")
        nc.gpsimd.memset(wsrc, 0.0)

        # small consts on the gpsimd SWDGE ring, in parallel with the above
        ones1 = cp.tile([1, P], F16, name="ones1", tag="ones1")
        nc.gpsimd.dma_start(ones1, ap["ones1"])
        bv = cp.tile([1, HPC * VW], F16, name="bv", tag="bv")
        nc.gpsimd.dma_start(bv, ap["bv"])
        tri = cp.tile([P, P], F16, name="tri", tag="tri")
        nc.gpsimd.dma_start(tri, ap["tri"])

        # ---- PE warmup: dense dummy matmuls while input DMAs stream in.
        # The PE clock-gate (HAM) unthrottles 1.2->2.4 GHz only after ~3.4us
        # of sustained matmul activity; burn that in on scratch data.
        # 26 matmuls cover the ~7us sync-queue preamble + the first weight/x
        # transfers at the cold (1.2 GHz) rate, so the PE is warm and fed
        # when the first real matmul's inputs land.
        wps = psB.tile([P, SQ], F32, name="wps", tag="acc")
        for i in range(19):
            nc.tensor.matmul(
                wps, wsrc[:, 0:P], wsrc, start=(i == 0), stop=(i == 18),
            )

        xts_k = [xts[:, k, :] for k in range(KT)]
        wq_t = [wq[:, k, :] for k in range(KT)]
        wk_t = [wk[:, k, :] for k in range(KT)]
        wv_t = [wv[:, k, :] for k in range(KT)]
        wp_t = [wp[:, k, :] for k in range(2)]

        # ---- activations living across phases ----
        qT = acts.tile([P, 2, S], F16, name="qT", tag="qT")
        kTt = acts.tile([P, 2, S], F16, name="kT", tag="kT")
        vv = acts.tile([P, NST, HPC * VW], F16, name="vv", tag="vv")
        outT = [acts.tile([P, S], F16, name=f"oT{i}", tag=f"oT{i}") for i in range(2)]

        def qk_pair(c, dst, wt, bias, pool):
            """Both i-halves of a q or k projection chunk, as interleaved
            chains into two PSUM regions (psC: one 2-bank tile; psA: two
            1-bank tiles) so consecutive accumulates hit different banks."""
            if pool is psC:
                ps2 = psC.tile([P, 2, SQ], F32, name="sc2", tag="sc")
                pss = [ps2[:, 0, :], ps2[:, 1, :]]
            else:
                pss = [psA.tile([P, SQ], F32, name="ps", tag="ps")
                       for _ in range(2)]
            for k in range(KT):
                for i in range(2):
                    nc.tensor.matmul(
                        pss[i],
                        wt[k][:, i * P:(i + 1) * P],
                        xts_k[k][:, c * SQ:(c + 1) * SQ],
                        start=(k == 0),
                        stop=(k == KT - 1),
                    )
            with nc.allow_low_precision(reason="fp16 matmul inputs"):
                if pool is psC:
                    nc.vector.tensor_add(
                        dst[:, :, c * SQ:(c + 1) * SQ], ps2,
                        bias[:, :, None].broadcast_to([P, 2, SQ]),
                    )
                else:
                    for i in range(2):
                        nc.vector.tensor_scalar_add(
                            dst[:, i, c * SQ:(c + 1) * SQ], pss[i],
                            bias[:, i:i + 1],
                        )

        def v_pair(st0, pool):
            """Two v seq-tiles as interleaved chains (natural layout +
            interleaved ones cols; the trailing ones matmul adds v-bias and
            the denominator ones column)."""
            if pool is psC:
                ps2 = psC.tile([P, 2, SQ], F32, name="sc2", tag="sc")
                pss = [ps2[:, 0, 0:HPC * VW], ps2[:, 1, 0:HPC * VW]]
            else:
                pss = [psA.tile([P, SQ], F32, name="psv", tag="ps")[:, 0:HPC * VW]
                       for _ in range(2)]
            for k in range(KT):
                for ci in range(2):
                    nc.tensor.matmul(
                        pss[ci],
                        xts_k[k][:, (st0 + ci) * P:(st0 + ci + 1) * P],
                        wv_t[k],
                        start=(k == 0),
                        stop=False,
                    )
            for ci in range(2):
                nc.tensor.matmul(pss[ci], ones1, bv, start=False, stop=True)
            with nc.allow_low_precision(reason="fp16 matmul inputs"):
                for ci in range(2):
                    nc.vector.tensor_copy(vv[:, st0 + ci, :], pss[ci])

        def qkv_groups(c, pool):
            yield lambda: qk_pair(c, qT, wq_t, bq, pool)
            yield lambda: qk_pair(c, kTt, wk_t, bk, pool)
            yield lambda: v_pair(4 * c, pool)
            yield lambda: v_pair(4 * c + 2, pool)

        def attention_pair(i, c, fillers=(), tail=False):
            """Heads 2i (kT/qT partition rows 0:64) and 2i+1 (rows 64:128).

            Both heads' scores for a key tile land in one 2-bank PSUM tile so
            a single exp instruction covers them (halves ScalarE instruction
            count). All scores are emitted before all attnv matmuls: the PE
            stream is in-order, so ScalarE's exps pipeline behind the score
            stream. The PE stalls on exp pacing both in the scores loop (sc
            PSUM rotation) and the attnv loop, so fillers pop on a schedule
            spread across BOTH loops. Diagonal blocks get their masked
            triangle zeroed by a GpSimd multiply with the tri mask."""
            nkt = 4 * (c + 1)
            accs = [psB.tile([VW, SQ], F32, name="acc", tag="acc")
                    for _ in range(2)]
            fillers = list(fillers)
            nf = len(fillers)
            steps = 2 * nkt

            def pop_due(step):
                while fillers and len(fillers) > nf * (steps - 1 - step) // steps:
                    fillers.pop(0)()

            exs = []
            for kt in range(nkt):
                colo = max(0, kt * P - c * SQ)
                diag = colo > 0 or kt * P == c * SQ
                sc2 = psC.tile([P, 2, SQ], F32, name="sc2", tag="sc")
                for j in range(2):
                    ro = j * 64
                    nc.tensor.matmul(
                        sc2[:, j, colo:SQ],
                        kTt[ro:ro + 64, i, kt * P:(kt + 1) * P],
                        qT[ro:ro + 64, i, c * SQ + colo:(c + 1) * SQ],
                        start=True,
                        stop=True,
                    )
                ex2 = exp_pool.tile([P, 2, SQ], F16, name="ex2", tag="ex")
                nc.scalar.activation(
                    ex2[:, :, colo:SQ], sc2[:, :, colo:SQ], AFT.Exp, scale=SCALE,
                )
                if diag:
                    # zero the masked triangle of the diagonal block on the
                    # (otherwise idle) GpSimd engine
                    nc.gpsimd.tensor_mul(
                        ex2[:, :, colo:colo + P],
                        ex2[:, :, colo:colo + P],
                        tri[:, None, :].broadcast_to([P, 2, P]),
                    )
                exs.append((ex2, kt, colo))
                # the scores loop is itself exp-paced (the sc PSUM slots
                # recycle only as ScalarE drains), so fillers pop here too
                if kt >= 2:
                    pop_due(kt - 2)
            for ex2, kt, colo in exs:
                for j in range(2):
                    h = 2 * i + j
                    nc.tensor.matmul(
                        accs[j][:, colo:SQ],
                        vv[:, kt, h * VW:(h + 1) * VW],
                        ex2[:, j, colo:SQ],
                        start=(kt == 0),
                        stop=(kt == nkt - 1),
                    )
                # dense PE filler between exp-paced attnv groups
                pop_due(nkt - 2 + kt)
            # the tiny denominator copies go first so the norm chain
            # (db matmuls -> reciprocal) starts as early as possible; at the
            # tail they run on ScalarE (idle once its last exp retires) so
            # they don't queue behind the filler ob copies on the DVE
            dns = []
            for j in range(2):
                dn = dh_pool.tile([1, SQ], F16, name="dn", tag="dn")
                with nc.allow_low_precision(reason="fp16 matmul inputs"):
                    if tail:
                        nc.scalar.copy(dn, accs[j][64:65, :])
                    else:
                        nc.vector.tensor_copy(dn, accs[j][64:65, :])
                dns.append(dn)
            for j in range(2):
                # at the tail ScalarE is idle: move the big outT copies there
                # so the DVE queue reaches the reciprocal/normalize sooner
                with nc.allow_low_precision(reason="fp16 matmul inputs"):
                    dst = outT[i][j * 64:j * 64 + 64, c * SQ:(c + 1) * SQ]
                    if tail:
                        nc.scalar.copy(dst, accs[j][0:64, :])
                    else:
                        nc.vector.tensor_copy(dst, accs[j][0:64, :])
            return dns

        def norm_pair(c, i, dns, pool=None):
            # outT *= 1/denominator: broadcast denoms via K=1 matmuls (the
            # two M=64 matmuls col-tile concurrently via base_partition),
            # one 128-lane fast reciprocal, one fp16 multiply
            db = (pool or psA).tile([P, SQ], F32, name="db",
                                    tag="acc" if pool is psB else "ps")
            nc.tensor.matmul(
                db[0:64, :], ones1[:, 0:64], dns[0],
                start=True, stop=True,
            )
            nc.tensor.matmul(
                db[64:P, :], ones1[:, 0:64], dns[1],
                start=True, stop=True,
            )
            rc32 = rc_pool.tile([P, SQ], F32, name="rc32", tag="rc32")
            nc.vector.reciprocal_approx_fast(rc32, db)
            with nc.allow_low_precision(reason="fp16 matmul inputs"):
                nc.vector.tensor_mul(
                    outT[i][:, c * SQ:(c + 1) * SQ],
                    outT[i][:, c * SQ:(c + 1) * SQ],
                    rc32,
                )

        def proj_mtile(m, tail=False):
            # one m-tile of the projection: [128 seq, 1024 outdims]; the two
            # nch contraction chains interleave across two PSUM tiles, and
            # both halves land in one ob tile -> a single 256KB out DMA
            ob = osb.tile([P, 2 * SQ], F16, name="ob", tag="ob")
            pss = [psA.tile([P, SQ], F32, name="ps", tag="ps")
                   for _ in range(2)]
            for kk in range(2):
                for nch in range(2):
                    nc.tensor.matmul(
                        pss[nch],
                        outT[kk][:, m * P:(m + 1) * P],
                        wp_t[kk][:, nch * SQ:(nch + 1) * SQ],
                        start=(kk == 0),
                        stop=(kk == 1),
                    )
            for nch in range(2):
                with nc.allow_low_precision(reason="partial sums; host sums fp32"):
                    # at the tail ScalarE is idle: split the two copies
                    # across ScalarE and DVE so they run concurrently
                    if tail and nch == 0:
                        nc.scalar.copy(ob[:, 0:SQ], pss[0])
                    else:
                        nc.vector.tensor_copy(
                            ob[:, nch * SQ:(nch + 1) * SQ], pss[nch])
                if tail:
                    # half-tile DMAs off alternating rings so the last
                    # transfer is small and starts as early as possible
                    eng = nc.gpsimd if nch == 0 else nc.sync
                    eng.dma_start(
                        ap["out"][m * P:(m + 1) * P, nch * SQ:(nch + 1) * SQ],
                        ob[:, nch * SQ:(nch + 1) * SQ])
            if not tail:
                nc.sync.dma_start(ap["out"][m * P:(m + 1) * P, :], ob)

        # ---- chunk-pipelined main body ----
        # chunk 0 QKV upfront through the score-PSUM pool (scores aren't
        # running yet), with warmup matmuls between the early groups to keep
        # the PE clock-gate busy while the x quarters stream in
        for gi, g in enumerate(qkv_groups(0, psC)):
            g()
            if gi < 2:
                for i in range(2):
                    nc.tensor.matmul(
                        wps, wsrc[:, 0:P], wsrc,
                        start=(i == 0), stop=(i == 1),
                    )
        # attention(c) runs against qkv chunks emitted one chunk ahead.
        # Filler slots between the exp-paced attnv groups carry, in order:
        # the previous pair's norm, the next chunk's qkv groups, and proj
        # m-tiles of chunks whose norms are complete. Chunk t's proj
        # (m=4t..4t+3) becomes eligible after norm(t,1).
        # per-pair filler budget tracks the pair's exp-pacing deficit
        # (~283ns x nkt + transition); proj tiles m0-m5 are deferred to the
        # last pair, which otherwise has no eligible work left and starves
        filler_plan = {
            (0, 0): [],
            (0, 1): ["n00"],
            (1, 0): ["n01"],
            (1, 1): ["n10"],
            (2, 0): ["n11"],
            (2, 1): ["n20"],
            (3, 0): ["n21", "m6", "m7", "m8", "m9", "m10", "m11"],
            (3, 1): ["n30", "m0", "m1", "m2", "m3", "m4", "m5"],
        }
        norms = {}          # (c, i) -> dns, filled as pairs complete

        def make_filler(tok):
            if tok.startswith("n"):
                c, i = int(tok[1]), int(tok[2])
                return lambda: norm_pair(c, i, norms[(c, i)])
            m = int(tok[1:])
            return lambda: proj_mtile(m)

        for c in range(NSQ):
            nxt = list(qkv_groups(c + 1, psA)) if c + 1 < NSQ else []
            for i in range(2):
                fillers = [make_filler(t) for t in filler_plan[(c, i)]]
                # next chunk's qkv interleaves after the norm filler
                fillers[1:1] = nxt[2 * i:2 * i + 2]
                norms[(c, i)] = attention_pair(
                    i, c, fillers, tail=(c == NSQ - 1 and i == 1))
        # tail: the last norm's broadcast goes through psB (psA would make
        # the first proj tile wait on the reciprocal), then chunk 3's proj
        norm_pair(NSQ - 1, 1, norms[(NSQ - 1, 1)], pool=psB)
        for m in range(12, 16):
            proj_mtile(m, tail=True)


def build_program():
    nc = bacc.Bacc("TRN2", target_bir_lowering=False, debug=False,
                   num_devices=NCORES)
    ap = {}
    for name, shape, dt in (
        ("xT", [P, NSQ, KT, SQ], F16), ("wq", [P, KT, CH], F16),
        ("wk", [P, KT, CH], F16), ("wv", [P, KT, HPC * VW], F16),
        ("bq", [P, 2], F32), ("bk", [P, 2], F32),
        ("bv", [1, HPC * VW], F16), ("wp", [P, 2, D], F16),
        ("tri", [P, P], F16), ("ones1", [1, P], F16),
    ):
        ap[name] = nc.dram_tensor(name, shape, dt, kind="ExternalInput").ap()
    ap["out"] = nc.dram_tensor("out", [S, D], F16, kind="ExternalOutput").ap()

    with tile.TileContext(nc) as tc:
        emit_kernel(nc, tc, ap)
    nc.compile()
    return nc


def make_core_inputs(hidden_states, w_attn, b_attn, w_proj):
    """Host-side sharding: per-core input dicts (core = batch*4 + head_group)."""
    f16, f32 = np.float16, np.float32
    x = np.asarray(hidden_states, f32)
    w_attn = np.asarray(w_attn, f32)
    b_attn = np.asarray(b_attn, f32)
    w_proj = np.asarray(w_proj, f32)

    tri = (np.arange(P)[:, None] <= np.arange(P)[None, :]).astype(f16)
    ones_row = np.ones((1, P), f16)

    def kmaj(w):
        # [D=(KT P), C] -> [P, KT, C] contiguous (the SBUF tile layout)
        return np.ascontiguousarray(
            w.reshape(KT, P, -1).transpose(1, 0, 2)).astype(f16)

    # x.T [(KT P), (NSQ SQ)] -> [P, NSQ, KT, SQ] contiguous seq-quarters
    xTs = [np.ascontiguousarray(
        x[b].T.reshape(KT, P, NSQ, SQ).transpose(1, 2, 0, 3)).astype(f16)
        for b in range(B)]

    in_maps = []
    for core in range(NCORES):
        b, g = core // HPC, core % HPC
        wq = kmaj(w_attn[:, g * CH:(g + 1) * CH])
        wk = kmaj(w_attn[:, D + g * CH:D + (g + 1) * CH])
        wv = np.zeros((D, HPC * VW), np.float32)
        bv = np.zeros((1, HPC * VW), f16)
        for h in range(HPC):
            src = 2 * D + (g * HPC + h) * HD
            wv[:, h * VW:h * VW + HD] = w_attn[:, src:src + HD]
            bv[0, h * VW:h * VW + HD] = b_attn[src:src + HD]
            bv[0, h * VW + HD] = 1.0
        wv = kmaj(wv)
        bq = np.ascontiguousarray(
            b_attn[g * CH:(g + 1) * CH].reshape(2, P).T)
        bk = np.ascontiguousarray(
            b_attn[D + g * CH:D + (g + 1) * CH].reshape(2, P).T)
        wp = np.ascontiguousarray(
            w_proj[g * CH:(g + 1) * CH, :].reshape(2, P, D)
            .transpose(1, 0, 2)).astype(f16)
        in_maps.append({
            "xT": xTs[b], "wq": wq, "wk": wk, "wv": wv,
            "bq": bq, "bk": bk, "bv": bv, "wp": wp,
            "tri": tri, "ones1": ones_row,
        })
    return in_maps


_PROGRAM = None


def kernel(hidden_states, w_attn, b_attn, w_proj, b_proj):
    global _PROGRAM
    if _PROGRAM is None:
        _PROGRAM = build_program()
    in_maps = make_core_inputs(hidden_states, w_attn, b_attn, w_proj)
    res = run_bass_kernel_spmd(_PROGRAM, in_maps, core_ids=list(range(NCORES)))
    out = np.zeros((B, S, D), np.float32)
    for core in range(NCORES):
        out[core // HPC] += res.results[core]["out"].astype(np.float32)
    out += np.asarray(b_proj, np.float32)
    return out


# revision 32
# speedup vs baseline: 1.2766x; 1.0091x over previous
"""Fused causal multi-head attention block on 8 Trainium2 NeuronCores.

Problem (GPT-2 style attention, B=2, S=2048, D=1024, H=16, hd=64):
    qkv = x @ w_attn + b_attn ; split q,k,v ; per-head causal softmax(q k^T / 8) v
    out = attn_out @ w_proj + b_proj

Sharding: data parallel on batch (2) x tensor parallel on heads (4 groups of 4
heads). Core c -> batch c//4, head group c%4. Each core computes a partial
[S, D] output (its heads' slice of w_proj rows); host sums the 4 partials per
batch and adds b_proj.

Per-core kernel layout tricks:
- scores are computed TRANSPOSED (scoresT[key, query]) so the softmax
  denominator falls out of the attn@v matmul by appending a ones-column to v:
  [v | 1]^T @ exp(scoresT) yields the unnormalized output and the per-query
  denominator in one PSUM accumulation.
- matmul inputs are fp16 (full PE rate + fast weight loads); all accumulation
  is fp32 in PSUM. exp(s/8) is in [0, ~13], well inside fp16 range.
- back-to-back matmuls accumulating into the SAME PSUM region serialize on
  the array drain (~175ns each). All contraction chains (qkv, proj) are
  emitted as interleaved PAIRS of chains targeting two PSUM regions, so one
  chain's fill overlaps the other's drain.
- causal masking: fully-masked blocks are skipped via restricted matmul
  widths; diagonal blocks get their masked triangle zeroed on the (otherwise
  idle) GpSimd engine, keeping both the PE and the vector engine out of the
  score->attnv chain.
- x is DMA'd in 4 seq-quarters so QKV chunk c only waits for quarter c; all
  bulk transfers ride one sync-ring queue in exact consumption order (the
  SDMA engines round-robin rings with no priority, so a second ring would
  steal bandwidth from the gating transfers). Small consts go out on the
  gpsimd SWDGE ring. Host pre-arranges every source in its SBUF layout so
  transfers run contiguous at line rate.
- emission is chunk-pipelined: QKV chunk c+1, proj of completed chunks, and
  the previous pair's normalization are spread as PE filler across BOTH the
  score loop (which is exp-paced through the sc-PSUM rotation) and the
  exp-paced attnv groups, sized to each pair's pacing deficit, with proj
  m0-m5 held back for the final pair which otherwise starves.
- the two per-pair score matmuls (K=64 at partition bases 0/64) row-tile
  into the PE array concurrently (verified: second matmul starts 4ns after
  the first); the two norm broadcast matmuls (M=64 at output bases 0/64)
  col-tile concurrently.
"""

import sys

sys.path.insert(0, "/opt/trn_rl_repo")

import numpy as np

import concourse.bass as bass
import concourse.mybir as mybir
import concourse.tile as tile
from concourse import bacc
from concourse.bass_utils import run_bass_kernel_spmd

F32 = mybir.dt.float32
F16 = mybir.dt.float16
AFT = mybir.ActivationFunctionType

B, S, D, H, HD = 2, 2048, 1024, 16, 64
NCORES = 8
HPC = 4            # heads per core
CH = HPC * HD      # 256 channels per core
VW = HD + 1        # v width incl. ones column
P = 128
KT = D // P        # 8 contraction tiles over D
SQ = 512           # query/N chunk
NSQ = S // SQ      # 4
NST = S // P       # 16 seq tiles
SCALE = 1.0 / np.sqrt(HD)


def emit_kernel(nc, tc, ap):
    """Emit the per-core program. `ap` is a dict of DRAM APs."""
    with (
        tc.tile_pool(name="const", bufs=1) as cp,
        tc.tile_pool(name="xw", bufs=1) as xw,
        tc.tile_pool(name="act", bufs=1) as acts,
        tc.tile_pool(name="ex", bufs=16) as exp_pool,
        tc.tile_pool(name="dh", bufs=4) as dh_pool,
        tc.tile_pool(name="rc", bufs=2) as rc_pool,
        tc.tile_pool(name="osb", bufs=3) as osb,
        tc.tile_pool(name="psA", bufs=2, space="PSUM") as psA,
        tc.tile_pool(name="psB", bufs=2, space="PSUM") as psB,
        tc.tile_pool(name="psC", bufs=2, space="PSUM") as psC,
    ):
        # ---- weight/x DMAs on the sync (HWDGE) ring, in consumption order.
        # All sources are host-prepared in the exact SBUF layout (contiguous
        # per-partition blocks) so the DMAs run at line rate. x comes in 4
        # seq-quarters so QKV chunk c is gated only on quarter c.
        # everything streams on the single sync HWDGE ring in consumption
        # order: the SDMA engines round-robin across rings with no priority,
        # so a second ring would steal HBM bandwidth from the gating
        # transfers at the front of this one
        # wq + x quarter 0 gate the first real matmul; both are split in
        # k-halves so the first half of the first contraction chain can
        # start after 0.75MB instead of 1.5MB. wk/bq/bk follow.
        wq = xw.tile([P, KT, CH], F16, name="wq", tag="wq")
        xts = xw.tile([P, KT, S], F16, name="xts", tag="xts")
        hk = KT // 2
        nc.sync.dma_start(wq[:, 0:hk, :], ap["wq"][:, 0:hk, :])
        nc.sync.dma_start(xts[:, 0:hk, 0:SQ], ap["xT"][:, 0, 0:hk, :])
        nc.sync.dma_start(wq[:, hk:KT, :], ap["wq"][:, hk:KT, :])
        nc.sync.dma_start(xts[:, hk:KT, 0:SQ], ap["xT"][:, 0, hk:KT, :])
        wk = xw.tile([P, KT, CH], F16, name="wk", tag="wk")
        nc.sync.dma_start(wk, ap["wk"])
        bq = cp.tile([P, 2], F32, name="bq", tag="bq")
        nc.sync.dma_start(bq, ap["bq"])
        bk = cp.tile([P, 2], F32, name="bk", tag="bk")
        nc.sync.dma_start(bk, ap["bk"])
        wv = xw.tile([P, KT, HPC * VW], F16, name="wv", tag="wv")
        nc.sync.dma_start(wv, ap["wv"])
        for c in range(1, NSQ):
            nc.sync.dma_start(xts[:, :, c * SQ:(c + 1) * SQ], ap["xT"][:, c])
        wp = xw.tile([P, 2, D], F16, name="wp", tag="wp")
        nc.sync.dma_start(wp, ap["wp"])

        # warmup scratch zeroed on gpsimd BEFORE its const DMAs queue up
        # (and not on the vector engine, whose preamble lands ~5us in)
        wsrc = cp.tile([P, SQ], F16, name="wsrc", tag="wsrc")
        nc.gpsimd.memset(wsrc, 0.0)

        # small consts on the gpsimd SWDGE ring, in parallel with the above
        ones1 = cp.tile([1, P], F16, name="ones1", tag="ones1")
        nc.gpsimd.dma_start(ones1, ap["ones1"])
        bv = cp.tile([1, HPC * VW], F16, name="bv", tag="bv")
        nc.gpsimd.dma_start(bv, ap["bv"])
        tri = cp.tile([P, P], F16, name="tri", tag="tri")
        nc.gpsimd.dma_start(tri, ap["tri"])

        # ---- PE warmup: dense dummy matmuls while input DMAs stream in.
        # The PE clock-gate (HAM) unthrottles 1.2->2.4 GHz only after ~3.4us
        # of sustained matmul activity; burn that in on scratch data.
        # 26 matmuls cover the ~7us sync-queue preamble + the first weight/x
        # transfers at the cold (1.2 GHz) rate, so the PE is warm and fed
        # when the first real matmul's inputs land.
        wps = psB.tile([P, SQ], F32, name="wps", tag="acc")
        for i in range(19):
            nc.tensor.matmul(
                wps, wsrc[:, 0:P], wsrc, start=(i == 0), stop=(i == 18),
            )

        xts_k = [xts[:, k, :] for k in range(KT)]
        wq_t = [wq[:, k, :] for k in range(KT)]
        wk_t = [wk[:, k, :] for k in range(KT)]
        wv_t = [wv[:, k, :] for k in range(KT)]
        wp_t = [wp[:, k, :] for k in range(2)]

        # ---- activations living across phases ----
        qT = acts.tile([P, 2, S], F16, name="qT", tag="qT")
        kTt = acts.tile([P, 2, S], F16, name="kT", tag="kT")
        vv = acts.tile([P, NST, HPC * VW], F16, name="vv", tag="vv")
        outT = [acts.tile([P, S], F16, name=f"oT{i}", tag=f"oT{i}") for i in range(2)]

        def qk_pair(c, dst, wt, bias, pool):
            """Both i-halves of a q or k projection chunk, as interleaved
            chains into two PSUM regions (psC: one 2-bank tile; psA: two
            1-bank tiles) so consecutive accumulates hit different banks."""
            if pool is psC:
                ps2 = psC.tile([P, 2, SQ], F32, name="sc2", tag="sc")
                pss = [ps2[:, 0, :], ps2[:, 1, :]]
            else:
                pss = [psA.tile([P, SQ], F32, name="ps", tag="ps")
                       for _ in range(2)]
            for k in range(KT):
                for i in range(2):
                    nc.tensor.matmul(
                        pss[i],
                        wt[k][:, i * P:(i + 1) * P],
                        xts_k[k][:, c * SQ:(c + 1) * SQ],
                        start=(k == 0),
                        stop=(k == KT - 1),
                    )
            with nc.allow_low_precision(reason="fp16 matmul inputs"):
                if pool is psC:
                    nc.vector.tensor_add(
                        dst[:, :, c * SQ:(c + 1) * SQ], ps2,
                        bias[:, :, None].broadcast_to([P, 2, SQ]),
                    )
                else:
                    for i in range(2):
                        nc.vector.tensor_scalar_add(
                            dst[:, i, c * SQ:(c + 1) * SQ], pss[i],
                            bias[:, i:i + 1],
                        )

        def v_pair(st0, pool):
            """Two v seq-tiles as interleaved chains (natural layout +
            interleaved ones cols; the trailing ones matmul adds v-bias and
            the denominator ones column)."""
            if pool is psC:
                ps2 = psC.tile([P, 2, SQ], F32, name="sc2", tag="sc")
                pss = [ps2[:, 0, 0:HPC * VW], ps2[:, 1, 0:HPC * VW]]
            else:
                pss = [psA.tile([P, SQ], F32, name="psv", tag="ps")[:, 0:HPC * VW]
                       for _ in range(2)]
            for k in range(KT):
                for ci in range(2):
                    nc.tensor.matmul(
                        pss[ci],
                        xts_k[k][:, (st0 + ci) * P:(st0 + ci + 1) * P],
                        wv_t[k],
                        start=(k == 0),
                        stop=False,
                    )
            for ci in range(2):
                nc.tensor.matmul(pss[ci], ones1, bv, start=False, stop=True)
            with nc.allow_low_precision(reason="fp16 matmul inputs"):
                for ci in range(2):
                    nc.vector.tensor_copy(vv[:, st0 + ci, :], pss[ci])

        def qkv_groups(c, pool):
            yield lambda: qk_pair(c, qT, wq_t, bq, pool)
            yield lambda: qk_pair(c, kTt, wk_t, bk, pool)
            yield lambda: v_pair(4 * c, pool)
            yield lambda: v_pair(4 * c + 2, pool)

        def attention_pair(i, c, fillers=(), tail=False):
            """Heads 2i (kT/qT partition rows 0:64) and 2i+1 (rows 64:128).

            Both heads' scores for a key tile land in one 2-bank PSUM tile so
            a single exp instruction covers them (halves ScalarE instruction
            count). All scores are emitted before all attnv matmuls: the PE
            stream is in-order, so ScalarE's exps pipeline behind the score
            stream. The PE stalls on exp pacing both in the scores loop (sc
            PSUM rotation) and the attnv loop, so fillers pop on a schedule
            spread across BOTH loops. Diagonal blocks get their masked
            triangle zeroed by a GpSimd multiply with the tri mask."""
            nkt = 4 * (c + 1)
            accs = [psB.tile([VW, SQ], F32, name="acc", tag="acc")
                    for _ in range(2)]
            fillers = list(fillers)
            nf = len(fillers)
            steps = 2 * nkt

            def pop_due(step):
                # one step ahead of the linear schedule: filler PSUM->SBUF
                # copies then clear the DVE queue before the next pair's
                # attnv needs the accs slots back
                while fillers and len(fillers) > nf * (steps - 2 - step) // steps:
                    fillers.pop(0)()

            exs = []
            for kt in range(nkt):
                colo = max(0, kt * P - c * SQ)
                diag = colo > 0 or kt * P == c * SQ
                sc2 = psC.tile([P, 2, SQ], F32, name="sc2", tag="sc")
                for j in range(2):
                    ro = j * 64
                    nc.tensor.matmul(
                        sc2[:, j, colo:SQ],
                        kTt[ro:ro + 64, i, kt * P:(kt + 1) * P],
                        qT[ro:ro + 64, i, c * SQ + colo:(c + 1) * SQ],
                        start=True,
                        stop=True,
                    )
                ex2 = exp_pool.tile([P, 2, SQ], F16, name="ex2", tag="ex")
                nc.scalar.activation(
                    ex2[:, :, colo:SQ], sc2[:, :, colo:SQ], AFT.Exp, scale=SCALE,
                )
                if diag:
                    # zero the masked triangle of the diagonal block on the
                    # (otherwise idle) GpSimd engine
                    nc.gpsimd.tensor_mul(
                        ex2[:, :, colo:colo + P],
                        ex2[:, :, colo:colo + P],
                        tri[:, None, :].broadcast_to([P, 2, P]),
                    )
                exs.append((ex2, kt, colo))
                # the scores loop is itself exp-paced (the sc PSUM slots
                # recycle only as ScalarE drains), so fillers pop here too
                if kt >= 2:
                    pop_due(kt - 2)
            pop_due(nkt - 2)
            for ex2, kt, colo in exs:
                for j in range(2):
                    h = 2 * i + j
                    nc.tensor.matmul(
                        accs[j][:, colo:SQ],
                        vv[:, kt, h * VW:(h + 1) * VW],
                        ex2[:, j, colo:SQ],
                        start=(kt == 0),
                        stop=(kt == nkt - 1),
                    )
                # dense PE filler between exp-paced attnv groups
                pop_due(nkt - 2 + kt)
            # the tiny denominator copies go first so the norm chain
            # (db matmuls -> reciprocal) starts as early as possible; at the
            # tail they run on ScalarE (idle once its last exp retires) so
            # they don't queue behind the filler ob copies on the DVE
            dns = []
            for j in range(2):
                dn = dh_pool.tile([1, SQ], F16, name="dn", tag="dn")
                with nc.allow_low_precision(reason="fp16 matmul inputs"):
                    if tail:
                        nc.scalar.copy(dn, accs[j][64:65, :])
                    else:
                        nc.vector.tensor_copy(dn, accs[j][64:65, :])
                dns.append(dn)
            for j in range(2):
                # at the tail ScalarE is idle: move the big outT copies there
                # so the DVE queue reaches the reciprocal/normalize sooner
                with nc.allow_low_precision(reason="fp16 matmul inputs"):
                    dst = outT[i][j * 64:j * 64 + 64, c * SQ:(c + 1) * SQ]
                    if tail:
                        nc.scalar.copy(dst, accs[j][0:64, :])
                    else:
                        nc.vector.tensor_copy(dst, accs[j][0:64, :])
            return dns

        def norm_pair(c, i, dns, pool=None):
            # outT *= 1/denominator: broadcast denoms via K=1 matmuls (the
            # two M=64 matmuls col-tile concurrently via base_partition),
            # one 128-lane fast reciprocal, one fp16 multiply
            db = (pool or psA).tile([P, SQ], F32, name="db",
                                    tag="acc" if pool is psB else "ps")
            nc.tensor.matmul(
                db[0:64, :], ones1[:, 0:64], dns[0],
                start=True, stop=True,
            )
            nc.tensor.matmul(
                db[64:P, :], ones1[:, 0:64], dns[1],
                start=True, stop=True,
            )
            rc32 = rc_pool.tile([P, SQ], F32, name="rc32", tag="rc32")
            nc.vector.reciprocal_approx_fast(rc32, db)
            with nc.allow_low_precision(reason="fp16 matmul inputs"):
                nc.vector.tensor_mul(
                    outT[i][:, c * SQ:(c + 1) * SQ],
                    outT[i][:, c * SQ:(c + 1) * SQ],
                    rc32,
                )

        def proj_mtile(m, tail=False):
            # one m-tile of the projection: [128 seq, 1024 outdims]; the two
            # nch contraction chains interleave across two PSUM tiles, and
            # both halves land in one ob tile -> a single 256KB out DMA
            ob = osb.tile([P, 2 * SQ], F16, name="ob", tag="ob")
            pss = [psA.tile([P, SQ], F32, name="ps", tag="ps")
                   for _ in range(2)]
            for kk in range(2):
                for nch in range(2):
                    nc.tensor.matmul(
                        pss[nch],
                        outT[kk][:, m * P:(m + 1) * P],
                        wp_t[kk][:, nch * SQ:(nch + 1) * SQ],
                        start=(kk == 0),
                        stop=(kk == 1),
                    )
            for nch in range(2):
                with nc.allow_low_precision(reason="partial sums; host sums fp32"):
                    # at the tail ScalarE is idle: split the two copies
                    # across ScalarE and DVE so they run concurrently
                    if tail and nch == 0:
                        nc.scalar.copy(ob[:, 0:SQ], pss[0])
                    else:
                        nc.vector.tensor_copy(
                            ob[:, nch * SQ:(nch + 1) * SQ], pss[nch])
                if tail:
                    # half-tile DMAs off alternating rings so the last
                    # transfer is small and starts as early as possible
                    eng = nc.gpsimd if nch == 0 else nc.sync
                    eng.dma_start(
                        ap["out"][m * P:(m + 1) * P, nch * SQ:(nch + 1) * SQ],
                        ob[:, nch * SQ:(nch + 1) * SQ])
            if not tail:
                nc.sync.dma_start(ap["out"][m * P:(m + 1) * P, :], ob)

        # ---- chunk-pipelined main body ----
        # chunk 0 QKV upfront through the score-PSUM pool (scores aren't
        # running yet), with warmup matmuls between the early groups to keep
        # the PE clock-gate busy while the x quarters stream in
        for gi, g in enumerate(qkv_groups(0, psC)):
            g()
            if gi < 2:
                for i in range(2):
                    nc.tensor.matmul(
                        wps, wsrc[:, 0:P], wsrc,
                        start=(i == 0), stop=(i == 1),
                    )
        # attention(c) runs against qkv chunks emitted one chunk ahead.
        # Filler slots between the exp-paced attnv groups carry, in order:
        # the previous pair's norm, the next chunk's qkv groups, and proj
        # m-tiles of chunks whose norms are complete. Chunk t's proj
        # (m=4t..4t+3) becomes eligible after norm(t,1).
        # per-pair filler budget tracks the pair's exp-pacing deficit
        # (~283ns x nkt + transition); proj tiles m0-m5 are deferred to the
        # last pair, which otherwise has no eligible work left and starves
        filler_plan = {
            (0, 0): [],
            (0, 1): ["n00"],
            (1, 0): ["n01"],
            (1, 1): ["n10"],
            (2, 0): ["n11"],
            (2, 1): ["n20"],
            (3, 0): ["n21", "m6", "m7", "m8", "m9", "m10", "m11"],
            (3, 1): ["n30", "m0", "m1", "m2", "m3", "m4", "m5"],
        }
        norms = {}          # (c, i) -> dns, filled as pairs complete

        def make_filler(tok):
            if tok.startswith("n"):
                c, i = int(tok[1]), int(tok[2])
                return lambda: norm_pair(c, i, norms[(c, i)])
            m = int(tok[1:])
            return lambda: proj_mtile(m)

        for c in range(NSQ):
            nxt = list(qkv_groups(c + 1, psA)) if c + 1 < NSQ else []
            for i in range(2):
                fillers = [make_filler(t) for t in filler_plan[(c, i)]]
                # next chunk's qkv interleaves after the norm filler
                fillers[1:1] = nxt[2 * i:2 * i + 2]
                norms[(c, i)] = attention_pair(
                    i, c, fillers, tail=(c == NSQ - 1 and i == 1))
        # tail: the last norm's broadcast goes through psB (psA would make
        # the first proj tile wait on the reciprocal), then chunk 3's proj
        # m12's first contraction half runs in the (now free) score-PSUM
        # slots around the last norm, hiding the norm's DVE latency chain
        # under dense PE work; the second half lands after the normalize.
        t12 = [psC.tile([P, SQ], F32, name="t12", tag="sc") for _ in range(2)]
        for nch in range(2):
            nc.tensor.matmul(
                t12[nch],
                outT[0][:, 12 * P:13 * P],
                wp_t[0][:, nch * SQ:(nch + 1) * SQ],
                start=True, stop=False,
            )
        norm_pair(NSQ - 1, 1, norms[(NSQ - 1, 1)], pool=psB)
        ob12 = osb.tile([P, 2 * SQ], F16, name="ob12", tag="ob")
        for nch in range(2):
            nc.tensor.matmul(
                t12[nch],
                outT[1][:, 12 * P:13 * P],
                wp_t[1][:, nch * SQ:(nch + 1) * SQ],
                start=False, stop=True,
            )
            with nc.allow_low_precision(reason="partial sums; host sums fp32"):
                if nch == 0:
                    nc.scalar.copy(ob12[:, 0:SQ], t12[0])
                else:
                    nc.vector.tensor_copy(ob12[:, SQ:2 * SQ], t12[1])
            eng = nc.gpsimd if nch == 0 else nc.sync
            eng.dma_start(
                ap["out"][12 * P:13 * P, nch * SQ:(nch + 1) * SQ],
                ob12[:, nch * SQ:(nch + 1) * SQ])
        for m in range(13, 16):
            proj_mtile(m, tail=True)


def build_program():
    nc = bacc.Bacc("TRN2", target_bir_lowering=False, debug=False,
                   num_devices=NCORES)
    ap = {}
    for name, shape, dt in (
        ("xT", [P, NSQ, KT, SQ], F16), ("wq", [P, KT, CH], F16),
        ("wk", [P, KT, CH], F16), ("wv", [P, KT, HPC * VW], F16),
        ("bq", [P, 2], F32), ("bk", [P, 2], F32),
        ("bv", [1, HPC * VW], F16), ("wp", [P, 2, D], F16),
        ("tri", [P, P], F16), ("ones1", [1, P], F16),
    ):
        ap[name] = nc.dram_tensor(name, shape, dt, kind="ExternalInput").ap()
    ap["out"] = nc.dram_tensor("out", [S, D], F16, kind="ExternalOutput").ap()

    with tile.TileContext(nc) as tc:
        emit_kernel(nc, tc, ap)
    nc.compile()
    return nc


def make_core_inputs(hidden_states, w_attn, b_attn, w_proj):
    """Host-side sharding: per-core input dicts (core = batch*4 + head_group)."""
    f16, f32 = np.float16, np.float32
    x = np.asarray(hidden_states, f32)
    w_attn = np.asarray(w_attn, f32)
    b_attn = np.asarray(b_attn, f32)
    w_proj = np.asarray(w_proj, f32)

    tri = (np.arange(P)[:, None] <= np.arange(P)[None, :]).astype(f16)
    ones_row = np.ones((1, P), f16)

    def kmaj(w):
        # [D=(KT P), C] -> [P, KT, C] contiguous (the SBUF tile layout)
        return np.ascontiguousarray(
            w.reshape(KT, P, -1).transpose(1, 0, 2)).astype(f16)

    # x.T [(KT P), (NSQ SQ)] -> [P, NSQ, KT, SQ] contiguous seq-quarters
    xTs = [np.ascontiguousarray(
        x[b].T.reshape(KT, P, NSQ, SQ).transpose(1, 2, 0, 3)).astype(f16)
        for b in range(B)]

    in_maps = []
    for core in range(NCORES):
        b, g = core // HPC, core % HPC
        wq = kmaj(w_attn[:, g * CH:(g + 1) * CH])
        wk = kmaj(w_attn[:, D + g * CH:D + (g + 1) * CH])
        wv = np.zeros((D, HPC * VW), np.float32)
        bv = np.zeros((1, HPC * VW), f16)
        for h in range(HPC):
            src = 2 * D + (g * HPC + h) * HD
            wv[:, h * VW:h * VW + HD] = w_attn[:, src:src + HD]
            bv[0, h * VW:h * VW + HD] = b_attn[src:src + HD]
            bv[0, h * VW + HD] = 1.0
        wv = kmaj(wv)
        bq = np.ascontiguousarray(
            b_attn[g * CH:(g + 1) * CH].reshape(2, P).T)
        bk = np.ascontiguousarray(
            b_attn[D + g * CH:D + (g + 1) * CH].reshape(2, P).T)
        wp = np.ascontiguousarray(
            w_proj[g * CH:(g + 1) * CH, :].reshape(2, P, D)
            .transpose(1, 0, 2)).astype(f16)
        in_maps.append({
            "xT": xTs[b], "wq": wq, "wk": wk, "wv": wv,
            "bq": bq, "bk": bk, "bv": bv, "wp": wp,
            "tri": tri, "ones1": ones_row,
        })
    return in_maps


_PROGRAM = None


def kernel(hidden_states, w_attn, b_attn, w_proj, b_proj):
    global _PROGRAM
    if _PROGRAM is None:
        _PROGRAM = build_program()
    in_maps = make_core_inputs(hidden_states, w_attn, b_attn, w_proj)
    res = run_bass_kernel_spmd(_PROGRAM, in_maps, core_ids=list(range(NCORES)))
    out = np.zeros((B, S, D), np.float32)
    for core in range(NCORES):
        out[core // HPC] += res.results[core]["out"].astype(np.float32)
    out += np.asarray(b_proj, np.float32)
    return out


# revision 34
# speedup vs baseline: 1.2944x; 1.0139x over previous
"""Fused causal multi-head attention block on 8 Trainium2 NeuronCores.

Problem (GPT-2 style attention, B=2, S=2048, D=1024, H=16, hd=64):
    qkv = x @ w_attn + b_attn ; split q,k,v ; per-head causal softmax(q k^T / 8) v
    out = attn_out @ w_proj + b_proj

Sharding: data parallel on batch (2) x tensor parallel on heads (4 groups of 4
heads). Core c -> batch c//4, head group c%4. Each core computes a partial
[S, D] output (its heads' slice of w_proj rows); host sums the 4 partials per
batch and adds b_proj.

Per-core kernel layout tricks:
- scores are computed TRANSPOSED (scoresT[key, query]) so the softmax
  denominator falls out of the attn@v matmul by appending a ones-column to v:
  [v | 1]^T @ exp(scoresT) yields the unnormalized output and the per-query
  denominator in one PSUM accumulation.
- matmul inputs are fp16 (full PE rate + fast weight loads); all accumulation
  is fp32 in PSUM. exp(s/8) is in [0, ~13], well inside fp16 range.
- back-to-back matmuls accumulating into the SAME PSUM region serialize on
  the array drain (~175ns each). All contraction chains (qkv, proj) are
  emitted as interleaved PAIRS of chains targeting two PSUM regions, so one
  chain's fill overlaps the other's drain.
- causal masking: fully-masked blocks are skipped via restricted matmul
  widths; diagonal blocks get their masked triangle zeroed on the (otherwise
  idle) GpSimd engine, keeping both the PE and the vector engine out of the
  score->attnv chain.
- x is DMA'd in 4 seq-quarters so QKV chunk c only waits for quarter c; all
  bulk transfers ride one sync-ring queue in exact consumption order (the
  SDMA engines round-robin rings with no priority, so a second ring would
  steal bandwidth from the gating transfers). Small consts go out on the
  gpsimd SWDGE ring. Host pre-arranges every source in its SBUF layout so
  transfers run contiguous at line rate.
- emission is chunk-pipelined: QKV chunk c+1, proj of completed chunks, and
  the previous pair's normalization are spread as PE filler across BOTH the
  score loop (which is exp-paced through the sc-PSUM rotation) and the
  exp-paced attnv groups, sized to each pair's pacing deficit, with proj
  m0-m5 held back for the final pair which otherwise starves.
- the two per-pair score matmuls (K=64 at partition bases 0/64) row-tile
  into the PE array concurrently (verified: second matmul starts 4ns after
  the first); the two norm broadcast matmuls (M=64 at output bases 0/64)
  col-tile concurrently.
"""

import sys

sys.path.insert(0, "/opt/trn_rl_repo")

import numpy as np

import concourse.bass as bass
import concourse.mybir as mybir
import concourse.tile as tile
from concourse import bacc
from concourse.bass_utils import run_bass_kernel_spmd

F32 = mybir.dt.float32
F16 = mybir.dt.float16
AFT = mybir.ActivationFunctionType

B, S, D, H, HD = 2, 2048, 1024, 16, 64
NCORES = 8
HPC = 4            # heads per core
CH = HPC * HD      # 256 channels per core
VW = HD + 1        # v width incl. ones column
P = 128
KT = D // P        # 8 contraction tiles over D
SQ = 512           # query/N chunk
NSQ = S // SQ      # 4
NST = S // P       # 16 seq tiles
SCALE = 1.0 / np.sqrt(HD)


def emit_kernel(nc, tc, ap):
    """Emit the per-core program. `ap` is a dict of DRAM APs."""
    with (
        tc.tile_pool(name="const", bufs=1) as cp,
        tc.tile_pool(name="xw", bufs=1) as xw,
        tc.tile_pool(name="act", bufs=1) as acts,
        tc.tile_pool(name="ex", bufs=16) as exp_pool,
        tc.tile_pool(name="dh", bufs=4) as dh_pool,
        tc.tile_pool(name="rc", bufs=2) as rc_pool,
        tc.tile_pool(name="osb", bufs=3) as osb,
        tc.tile_pool(name="psA", bufs=2, space="PSUM") as psA,
        tc.tile_pool(name="psB", bufs=2, space="PSUM") as psB,
        tc.tile_pool(name="psC", bufs=2, space="PSUM") as psC,
    ):
        # ---- weight/x DMAs on the sync (HWDGE) ring, in consumption order.
        # All sources are host-prepared in the exact SBUF layout (contiguous
        # per-partition blocks) so the DMAs run at line rate. x comes in 4
        # seq-quarters so QKV chunk c is gated only on quarter c.
        # everything streams on the single sync HWDGE ring in consumption
        # order: the SDMA engines round-robin across rings with no priority,
        # so a second ring would steal HBM bandwidth from the gating
        # transfers at the front of this one
        # wq + x quarter 0 gate the first real matmul; both are split in
        # k-halves so the first half of the first contraction chain can
        # start after 0.75MB instead of 1.5MB. wk/bq/bk follow. (DMA issue
        # is only possible from sync/scalar/gpsimd; issuing these on a
        # second ring loses the bandwidth-priority the in-order sync ring
        # gives the gating transfers, so everything bulk stays here.)
        wq = xw.tile([P, KT, CH], F16, name="wq", tag="wq")
        xts = xw.tile([P, KT, S], F16, name="xts", tag="xts")
        hk = KT // 2
        nc.sync.dma_start(wq[:, 0:hk, :], ap["wq"][:, 0:hk, :])
        nc.sync.dma_start(xts[:, 0:hk, 0:SQ], ap["xT"][:, 0, 0:hk, :])
        nc.sync.dma_start(wq[:, hk:KT, :], ap["wq"][:, hk:KT, :])
        nc.sync.dma_start(xts[:, hk:KT, 0:SQ], ap["xT"][:, 0, hk:KT, :])
        wk = xw.tile([P, KT, CH], F16, name="wk", tag="wk")
        nc.sync.dma_start(wk, ap["wk"])
        bq = cp.tile([P, 2], F32, name="bq", tag="bq")
        nc.sync.dma_start(bq, ap["bq"])
        bk = cp.tile([P, 2], F32, name="bk", tag="bk")
        nc.sync.dma_start(bk, ap["bk"])
        wv = xw.tile([P, KT, HPC * VW], F16, name="wv", tag="wv")
        nc.sync.dma_start(wv, ap["wv"])
        for c in range(1, NSQ):
            nc.sync.dma_start(xts[:, :, c * SQ:(c + 1) * SQ], ap["xT"][:, c])
        wp = xw.tile([P, 2, D], F16, name="wp", tag="wp")
        nc.sync.dma_start(wp, ap["wp"])

        # warmup scratch zeroed on gpsimd BEFORE its const DMAs queue up
        # (and not on the vector engine, whose preamble lands ~5us in)
        wsrc = cp.tile([P, SQ], F16, name="wsrc", tag="wsrc")
        nc.gpsimd.memset(wsrc, 0.0)

        # small consts on the gpsimd SWDGE ring, in parallel with the above
        ones1 = cp.tile([1, P], F16, name="ones1", tag="ones1")
        nc.gpsimd.dma_start(ones1, ap["ones1"])
        bv = cp.tile([1, HPC * VW], F16, name="bv", tag="bv")
        nc.gpsimd.dma_start(bv, ap["bv"])
        tri = cp.tile([P, P], F16, name="tri", tag="tri")
        nc.gpsimd.dma_start(tri, ap["tri"])

        # ---- PE warmup: dense dummy matmuls while input DMAs stream in.
        # The PE clock-gate (HAM) unthrottles 1.2->2.4 GHz only after ~3.4us
        # of sustained matmul activity; burn that in on scratch data.
        # 26 matmuls cover the ~7us sync-queue preamble + the first weight/x
        # transfers at the cold (1.2 GHz) rate, so the PE is warm and fed
        # when the first real matmul's inputs land.
        wps = psB.tile([P, SQ], F32, name="wps", tag="acc")
        for i in range(19):
            nc.tensor.matmul(
                wps, wsrc[:, 0:P], wsrc, start=(i == 0), stop=(i == 18),
            )

        xts_k = [xts[:, k, :] for k in range(KT)]
        wq_t = [wq[:, k, :] for k in range(KT)]
        wk_t = [wk[:, k, :] for k in range(KT)]
        wv_t = [wv[:, k, :] for k in range(KT)]
        wp_t = [wp[:, k, :] for k in range(2)]

        # ---- activations living across phases ----
        qT = acts.tile([P, 2, S], F16, name="qT", tag="qT")
        kTt = acts.tile([P, 2, S], F16, name="kT", tag="kT")
        vv = acts.tile([P, NST, HPC * VW], F16, name="vv", tag="vv")
        outT = [acts.tile([P, S], F16, name=f"oT{i}", tag=f"oT{i}") for i in range(2)]

        def qk_pair(c, dst, wt, bias, pool):
            """Both i-halves of a q or k projection chunk, as interleaved
            chains into two PSUM regions (psC: one 2-bank tile; psA: two
            1-bank tiles) so consecutive accumulates hit different banks."""
            if pool is psC:
                ps2 = psC.tile([P, 2, SQ], F32, name="sc2", tag="sc")
                pss = [ps2[:, 0, :], ps2[:, 1, :]]
            else:
                pss = [psA.tile([P, SQ], F32, name="ps", tag="ps")
                       for _ in range(2)]
            for k in range(KT):
                for i in range(2):
                    nc.tensor.matmul(
                        pss[i],
                        wt[k][:, i * P:(i + 1) * P],
                        xts_k[k][:, c * SQ:(c + 1) * SQ],
                        start=(k == 0),
                        stop=(k == KT - 1),
                    )
            with nc.allow_low_precision(reason="fp16 matmul inputs"):
                if pool is psC:
                    nc.vector.tensor_add(
                        dst[:, :, c * SQ:(c + 1) * SQ], ps2,
                        bias[:, :, None].broadcast_to([P, 2, SQ]),
                    )
                else:
                    for i in range(2):
                        nc.vector.tensor_scalar_add(
                            dst[:, i, c * SQ:(c + 1) * SQ], pss[i],
                            bias[:, i:i + 1],
                        )

        def v_pair(st0, pool):
            """Two v seq-tiles as interleaved chains (natural layout +
            interleaved ones cols; the trailing ones matmul adds v-bias and
            the denominator ones column)."""
            if pool is psC:
                ps2 = psC.tile([P, 2, SQ], F32, name="sc2", tag="sc")
                pss = [ps2[:, 0, 0:HPC * VW], ps2[:, 1, 0:HPC * VW]]
            else:
                pss = [psA.tile([P, SQ], F32, name="psv", tag="ps")[:, 0:HPC * VW]
                       for _ in range(2)]
            for k in range(KT):
                for ci in range(2):
                    nc.tensor.matmul(
                        pss[ci],
                        xts_k[k][:, (st0 + ci) * P:(st0 + ci + 1) * P],
                        wv_t[k],
                        start=(k == 0),
                        stop=False,
                    )
            for ci in range(2):
                nc.tensor.matmul(pss[ci], ones1, bv, start=False, stop=True)
            with nc.allow_low_precision(reason="fp16 matmul inputs"):
                for ci in range(2):
                    nc.vector.tensor_copy(vv[:, st0 + ci, :], pss[ci])

        def qkv_groups(c, pool):
            yield lambda: qk_pair(c, qT, wq_t, bq, pool)
            yield lambda: qk_pair(c, kTt, wk_t, bk, pool)
            yield lambda: v_pair(4 * c, pool)
            yield lambda: v_pair(4 * c + 2, pool)

        def attention_pair(i, c, fillers=(), tail=False):
            """Heads 2i (kT/qT partition rows 0:64) and 2i+1 (rows 64:128).

            Both heads' scores for a key tile land in one 2-bank PSUM tile so
            a single exp instruction covers them (halves ScalarE instruction
            count). All scores are emitted before all attnv matmuls: the PE
            stream is in-order, so ScalarE's exps pipeline behind the score
            stream. The PE stalls on exp pacing both in the scores loop (sc
            PSUM rotation) and the attnv loop, so fillers pop on a schedule
            spread across BOTH loops. Diagonal blocks get their masked
            triangle zeroed by a GpSimd multiply with the tri mask."""
            nkt = 4 * (c + 1)
            accs = [psB.tile([VW, SQ], F32, name="acc", tag="acc")
                    for _ in range(2)]
            fillers = list(fillers)
            nf = len(fillers)
            steps = 2 * nkt

            def pop_due(step):
                # one step ahead of the linear schedule: filler PSUM->SBUF
                # copies then clear the DVE queue before the next pair's
                # attnv needs the accs slots back
                while fillers and len(fillers) > nf * (steps - 2 - step) // steps:
                    fillers.pop(0)()

            exs = []
            for kt in range(nkt):
                colo = max(0, kt * P - c * SQ)
                diag = colo > 0 or kt * P == c * SQ
                sc2 = psC.tile([P, 2, SQ], F32, name="sc2", tag="sc")
                for j in range(2):
                    ro = j * 64
                    nc.tensor.matmul(
                        sc2[:, j, colo:SQ],
                        kTt[ro:ro + 64, i, kt * P:(kt + 1) * P],
                        qT[ro:ro + 64, i, c * SQ + colo:(c + 1) * SQ],
                        start=True,
                        stop=True,
                    )
                ex2 = exp_pool.tile([P, 2, SQ], F16, name="ex2", tag="ex")
                nc.scalar.activation(
                    ex2[:, :, colo:SQ], sc2[:, :, colo:SQ], AFT.Exp, scale=SCALE,
                )
                if diag:
                    # zero the masked triangle of the diagonal block on the
                    # (otherwise idle) GpSimd engine
                    nc.gpsimd.tensor_mul(
                        ex2[:, :, colo:colo + P],
                        ex2[:, :, colo:colo + P],
                        tri[:, None, :].broadcast_to([P, 2, P]),
                    )
                exs.append((ex2, kt, colo))
                # the scores loop is itself exp-paced (the sc PSUM slots
                # recycle only as ScalarE drains), so fillers pop here too
                if kt >= 2:
                    pop_due(kt - 2)
            pop_due(nkt - 2)
            for ex2, kt, colo in exs:
                for j in range(2):
                    h = 2 * i + j
                    nc.tensor.matmul(
                        accs[j][:, colo:SQ],
                        vv[:, kt, h * VW:(h + 1) * VW],
                        ex2[:, j, colo:SQ],
                        start=(kt == 0),
                        stop=(kt == nkt - 1),
                    )
                # dense PE filler between exp-paced attnv groups
                pop_due(nkt - 2 + kt)
            # the tiny denominator copies go first so the norm chain
            # (db matmuls -> reciprocal) starts as early as possible; at the
            # tail they run on ScalarE (idle once its last exp retires) so
            # they don't queue behind the filler ob copies on the DVE
            dns = []
            for j in range(2):
                dn = dh_pool.tile([1, SQ], F16, name="dn", tag="dn")
                with nc.allow_low_precision(reason="fp16 matmul inputs"):
                    if tail:
                        nc.scalar.copy(dn, accs[j][64:65, :])
                    else:
                        nc.vector.tensor_copy(dn, accs[j][64:65, :])
                dns.append(dn)
            for j in range(2):
                # at the tail ScalarE is idle: move the big outT copies there
                # so the DVE queue reaches the reciprocal/normalize sooner
                with nc.allow_low_precision(reason="fp16 matmul inputs"):
                    dst = outT[i][j * 64:j * 64 + 64, c * SQ:(c + 1) * SQ]
                    if tail:
                        nc.scalar.copy(dst, accs[j][0:64, :])
                    else:
                        nc.vector.tensor_copy(dst, accs[j][0:64, :])
            return dns

        def norm_pair(c, i, dns, pool=None):
            # outT *= 1/denominator: broadcast denoms via K=1 matmuls (the
            # two M=64 matmuls col-tile concurrently via base_partition),
            # one 128-lane fast reciprocal, one fp16 multiply
            db = (pool or psA).tile([P, SQ], F32, name="db",
                                    tag="acc" if pool is psB else "ps")
            nc.tensor.matmul(
                db[0:64, :], ones1[:, 0:64], dns[0],
                start=True, stop=True,
            )
            nc.tensor.matmul(
                db[64:P, :], ones1[:, 0:64], dns[1],
                start=True, stop=True,
            )
            rc32 = rc_pool.tile([P, SQ], F32, name="rc32", tag="rc32")
            nc.vector.reciprocal_approx_fast(rc32, db)
            with nc.allow_low_precision(reason="fp16 matmul inputs"):
                nc.vector.tensor_mul(
                    outT[i][:, c * SQ:(c + 1) * SQ],
                    outT[i][:, c * SQ:(c + 1) * SQ],
                    rc32,
                )

        def proj_mtile(m, tail=False):
            # one m-tile of the projection: [128 seq, 1024 outdims]; the two
            # nch contraction chains interleave across two PSUM tiles, and
            # both halves land in one ob tile -> a single 256KB out DMA
            ob = osb.tile([P, 2 * SQ], F16, name="ob", tag="ob")
            pss = [psA.tile([P, SQ], F32, name="ps", tag="ps")
                   for _ in range(2)]
            for kk in range(2):
                for nch in range(2):
                    nc.tensor.matmul(
                        pss[nch],
                        outT[kk][:, m * P:(m + 1) * P],
                        wp_t[kk][:, nch * SQ:(nch + 1) * SQ],
                        start=(kk == 0),
                        stop=(kk == 1),
                    )
            for nch in range(2):
                with nc.allow_low_precision(reason="partial sums; host sums fp32"):
                    # at the tail ScalarE is idle: split the two copies
                    # across ScalarE and DVE so they run concurrently
                    if tail and nch == 0:
                        nc.scalar.copy(ob[:, 0:SQ], pss[0])
                    else:
                        nc.vector.tensor_copy(
                            ob[:, nch * SQ:(nch + 1) * SQ], pss[nch])
                if tail:
                    # half-tile DMAs off alternating rings so the last
                    # transfer is small and starts as early as possible
                    eng = nc.gpsimd if nch == 0 else nc.sync
                    eng.dma_start(
                        ap["out"][m * P:(m + 1) * P, nch * SQ:(nch + 1) * SQ],
                        ob[:, nch * SQ:(nch + 1) * SQ])
            if not tail:
                nc.sync.dma_start(ap["out"][m * P:(m + 1) * P, :], ob)

        # ---- chunk-pipelined main body ----
        # chunk 0 QKV upfront through the score-PSUM pool (scores aren't
        # running yet), with warmup matmuls between the early groups to keep
        # the PE clock-gate busy while the x quarters stream in
        for gi, g in enumerate(qkv_groups(0, psC)):
            g()
            if gi < 2:
                for i in range(2):
                    nc.tensor.matmul(
                        wps, wsrc[:, 0:P], wsrc,
                        start=(i == 0), stop=(i == 1),
                    )
        # attention(c) runs against qkv chunks emitted one chunk ahead.
        # Filler slots between the exp-paced attnv groups carry, in order:
        # the previous pair's norm, the next chunk's qkv groups, and proj
        # m-tiles of chunks whose norms are complete. Chunk t's proj
        # (m=4t..4t+3) becomes eligible after norm(t,1).
        # per-pair filler budget tracks the pair's exp-pacing deficit
        # (~283ns x nkt + transition); proj tiles m0-m5 are deferred to the
        # last pair, which otherwise has no eligible work left and starves
        filler_plan = {
            (0, 0): [],
            (0, 1): ["n00"],
            (1, 0): ["n01"],
            (1, 1): ["n10"],
            (2, 0): ["n11"],
            (2, 1): ["n20"],
            (3, 0): ["n21", "m6", "m7", "m8", "m9", "m10", "m11"],
            (3, 1): ["n30", "m0", "m1", "m2", "m3", "m4", "m5"],
        }
        norms = {}          # (c, i) -> dns, filled as pairs complete

        def make_filler(tok):
            if tok.startswith("n"):
                c, i = int(tok[1]), int(tok[2])
                return lambda: norm_pair(c, i, norms[(c, i)])
            m = int(tok[1:])
            return lambda: proj_mtile(m)

        for c in range(NSQ):
            nxt = list(qkv_groups(c + 1, psA)) if c + 1 < NSQ else []
            for i in range(2):
                fillers = [make_filler(t) for t in filler_plan[(c, i)]]
                # next chunk's qkv interleaves after the norm filler
                fillers[1:1] = nxt[2 * i:2 * i + 2]
                norms[(c, i)] = attention_pair(
                    i, c, fillers, tail=(c == NSQ - 1 and i == 1))
        # tail: the last norm's broadcast goes through psB (psA would make
        # the first proj tile wait on the reciprocal), then chunk 3's proj
        # m12's first contraction half runs in the (now free) score-PSUM
        # slots around the last norm, hiding the norm's DVE latency chain
        # under dense PE work; the second half lands after the normalize.
        t12 = [psC.tile([P, SQ], F32, name="t12", tag="sc") for _ in range(2)]
        for nch in range(2):
            nc.tensor.matmul(
                t12[nch],
                outT[0][:, 12 * P:13 * P],
                wp_t[0][:, nch * SQ:(nch + 1) * SQ],
                start=True, stop=False,
            )
        norm_pair(NSQ - 1, 1, norms[(NSQ - 1, 1)], pool=psB)
        ob12 = osb.tile([P, 2 * SQ], F16, name="ob12", tag="ob")
        for nch in range(2):
            nc.tensor.matmul(
                t12[nch],
                outT[1][:, 12 * P:13 * P],
                wp_t[1][:, nch * SQ:(nch + 1) * SQ],
                start=False, stop=True,
            )
            with nc.allow_low_precision(reason="partial sums; host sums fp32"):
                if nch == 0:
                    nc.scalar.copy(ob12[:, 0:SQ], t12[0])
                else:
                    nc.vector.tensor_copy(ob12[:, SQ:2 * SQ], t12[1])
            eng = nc.gpsimd if nch == 0 else nc.sync
            eng.dma_start(
                ap["out"][12 * P:13 * P, nch * SQ:(nch + 1) * SQ],
                ob12[:, nch * SQ:(nch + 1) * SQ])
        for m in range(13, 16):
            proj_mtile(m, tail=True)


def build_program():
    nc = bacc.Bacc("TRN2", target_bir_lowering=False, debug=False,
                   num_devices=NCORES)
    ap = {}
    for name, shape, dt in (
        ("xT", [P, NSQ, KT, SQ], F16), ("wq", [P, KT, CH], F16),
        ("wk", [P, KT, CH], F16), ("wv", [P, KT, HPC * VW], F16),
        ("bq", [P, 2], F32), ("bk", [P, 2], F32),
        ("bv", [1, HPC * VW], F16), ("wp", [P, 2, D], F16),
        ("tri", [P, P], F16), ("ones1", [1, P], F16),
    ):
        ap[name] = nc.dram_tensor(name, shape, dt, kind="ExternalInput").ap()
    ap["out"] = nc.dram_tensor("out", [S, D], F16, kind="ExternalOutput").ap()

    with tile.TileContext(nc) as tc:
        emit_kernel(nc, tc, ap)
    nc.compile()
    return nc


def make_core_inputs(hidden_states, w_attn, b_attn, w_proj):
    """Host-side sharding: per-core input dicts (core = batch*4 + head_group)."""
    f16, f32 = np.float16, np.float32
    x = np.asarray(hidden_states, f32)
    w_attn = np.asarray(w_attn, f32)
    b_attn = np.asarray(b_attn, f32)
    w_proj = np.asarray(w_proj, f32)

    tri = (np.arange(P)[:, None] <= np.arange(P)[None, :]).astype(f16)
    ones_row = np.ones((1, P), f16)

    def kmaj(w):
        # [D=(KT P), C] -> [P, KT, C] contiguous (the SBUF tile layout)
        return np.ascontiguousarray(
            w.reshape(KT, P, -1).transpose(1, 0, 2)).astype(f16)

    # x.T [(KT P), (NSQ SQ)] -> [P, NSQ, KT, SQ] contiguous seq-quarters
    xTs = [np.ascontiguousarray(
        x[b].T.reshape(KT, P, NSQ, SQ).transpose(1, 2, 0, 3)).astype(f16)
        for b in range(B)]

    in_maps = []
    for core in range(NCORES):
        b, g = core // HPC, core % HPC
        wq = kmaj(w_attn[:, g * CH:(g + 1) * CH])
        wk = kmaj(w_attn[:, D + g * CH:D + (g + 1) * CH])
        wv = np.zeros((D, HPC * VW), np.float32)
        bv = np.zeros((1, HPC * VW), f16)
        for h in range(HPC):
            src = 2 * D + (g * HPC + h) * HD
            wv[:, h * VW:h * VW + HD] = w_attn[:, src:src + HD]
            bv[0, h * VW:h * VW + HD] = b_attn[src:src + HD]
            bv[0, h * VW + HD] = 1.0
        wv = kmaj(wv)
        bq = np.ascontiguousarray(
            b_attn[g * CH:(g + 1) * CH].reshape(2, P).T)
        bk = np.ascontiguousarray(
            b_attn[D + g * CH:D + (g + 1) * CH].reshape(2, P).T)
        wp = np.ascontiguousarray(
            w_proj[g * CH:(g + 1) * CH, :].reshape(2, P, D)
            .transpose(1, 0, 2)).astype(f16)
        in_maps.append({
            "xT": xTs[b], "wq": wq, "wk": wk, "wv": wv,
            "bq": bq, "bk": bk, "bv": bv, "wp": wp,
            "tri": tri, "ones1": ones_row,
        })
    return in_maps


_PROGRAM = None


def kernel(hidden_states, w_attn, b_attn, w_proj, b_proj):
    global _PROGRAM
    if _PROGRAM is None:
        _PROGRAM = build_program()
    in_maps = make_core_inputs(hidden_states, w_attn, b_attn, w_proj)
    res = run_bass_kernel_spmd(_PROGRAM, in_maps, core_ids=list(range(NCORES)))
    out = np.zeros((B, S, D), np.float32)
    for core in range(NCORES):
        out[core // HPC] += res.results[core]["out"].astype(np.float32)
    out += np.asarray(b_proj, np.float32)
    return out
